# revision 39
# baseline (speedup 1.0000x reference)
"""Tree-GRU (arity-8, depth-5) over embedded leaves on 8 TRN2 NeuronCores.

Sharding: data-parallel over subtrees. Each core takes 4096 contiguous leaves
and runs the two large tree levels locally (512 -> 64 parents). The three tiny
tail levels (8 -> 1 parents per core, plus the root across cores) are
latency-bound chains of 8-wide GRU steps over <=73 nodes total; they run on
host from the shipped level-4 outputs (the sharding hint's "all-gather the
last log2(M) levels" relaxation).

The leaf level never materializes embeddings on device: the input-gate
projection G = embed_table @ w_ih.T (+ folded biases) is precomputed on host
(token-independent weight prep, like the weight retiling) and the kernel
gathers G rows per leaf with dma_gather(transpose=True), which lands the rows
directly in feature-major layout [128, 9, leaves] -- no PE transposes and no
on-device input-gate matmuls for the leaf level. Tokens are pre-permuted
child-major so each gather covers exactly one (step, chunk)'s children;
gathers fan out over 4 SWDGE queues and the gpsimd queue carries nothing else
until the last gather issues.

Per step the r/z pre-activations accumulate in one 3-bank PSUM tile: gathered
gi tiles are injected with identity matmuls (region-ordered start=True bank
clears), recurrent h @ w_hh.T tiles accumulate on top, and a single sigmoid
reads the whole tile. The n-role pre-activation gets b_hh_n from a one-hot
bias matmul (the hn bank's start=True writer), so the gate chain is one
bf16 multiply and one bf16 add before the tanh. Biases ride in G (b_ih +
b_hh for r/z, b_ih for n). Hidden states live in a 3-deep ring of per-step
tiles and both hidden accumulators are parent-major, so the (slow,
late-starting) gpsimd accumulator adds run contiguous and off the chain.
"""

import numpy as np
import ml_dtypes

ARITY = 8
DIM = 384
VOCAB = 32000
NCORES = 8
P = 128
J = 3  # DIM // 128 feature tiles
N_LEAVES = 32768
LEAVES_CORE = N_LEAVES // NCORES  # 4096
P5 = LEAVES_CORE // ARITY  # 512 level-5 parents per core
P4 = P5 // ARITY  # 64 level-4 parents per core
NCH = 256  # level-5 chunk size (2 chunks)

BF16 = ml_dtypes.bfloat16

_PROG_CACHE = {}


def _emit(tc, nc, aps):
    import concourse.mybir as mybir

    f32 = mybir.dt.float32
    bf16 = mybir.dt.bfloat16
    i16 = mybir.dt.int16
    Sig = mybir.ActivationFunctionType.Sigmoid
    Tanh = mybir.ActivationFunctionType.Tanh
    Add = mybir.AluOpType.add
    Sub = mybir.AluOpType.subtract
    Mult = mybir.AluOpType.mult

    (gtab, idxs, ident_in, whh_t, wih_s, biases, threes, out_x, out_h) = aps

    from contextlib import ExitStack

    with ExitStack() as ctx:
        const = ctx.enter_context(tc.tile_pool(name="const", bufs=1))
        gpool = ctx.enter_context(tc.tile_pool(name="gpool", bufs=1))
        state = ctx.enter_context(tc.tile_pool(name="state", bufs=1))
        gates = ctx.enter_context(tc.tile_pool(name="gates", bufs=3))
        pspool = ctx.enter_context(tc.tile_pool(name="pspool", bufs=1, space="PSUM"))

        # ---- prologue: idxs first, then the 16 transposing half-gathers ----
        # ONE idx DMA: with a single completion semaphore the first gather's
        # wait stays inline on the gather op, so the auto-inserted gather-lib
        # load (~13us ucode fetch) runs ahead of the wait instead of behind
        # an idx-wait hoisted in front of it.
        idx_sb = const.tile([P, 2 * ARITY, NCH // 16], i16)
        nc.scalar.dma_start(idx_sb[:], idxs[:])

        gi_tiles = []  # [t][ch] -> [128, 9, 256]
        for g in range(ARITY):
            halves = []
            for h in range(2):
                gi = gpool.tile([P, 9, NCH], bf16, name=f"gi{g}_{h}", tag=f"gi{g}_{h}")
                nc.gpsimd.dma_gather(
                    out_ap=gi[:],
                    in_ap=gtab[:],
                    idxs_ap=idx_sb[:, 2 * g + h, :],
                    num_idxs=NCH,
                    num_idxs_reg=NCH,
                    elem_size=9 * P,
                    transpose=True,
                    queue_num=(2 * g + h) % 4,
                )
                halves.append(gi)
            gi_tiles.append(halves)

        # ---- constants / weights (overlap with gathers) ----
        # 7 DMAs total (2 idx + 5 here) so no completion-semaphore reuse.
        # The [3, x] matmul constants ride in one packed blob.
        whh_sb = const.tile([P, J, 9, P], bf16)
        wih_sb = const.tile([P, J, 9, P], bf16)
        bias_sb = const.tile([P, 12], f32)
        threes_sb = const.tile([3, 3008], bf16)
        biasmm_sb = threes_sb[:, 0:512].rearrange("p (a n) -> p a n", a=4)
        onehot_sb = threes_sb[:, 512:1280].rearrange("p (a n) -> p a n", a=3)
        onehot512_sb = threes_sb[:, 1280:2816].rearrange("p (a n) -> p a n", a=3)
        onehot4_sb = threes_sb[:, 2816:3008].rearrange("p (a n) -> p a n", a=3)
        # ident blob: [:, 0:128] identity, [:, 128:896] b_hh_n broadcast over
        # NCH cols per m-tile (rhs of the hn bias id-MMs -- reuses the
        # already-loaded identity stationary, no biasmm LDWEIGHTS)
        identb = const.tile([P, P + 3 * NCH], bf16)
        ident = identb[:, 0:P]
        bvec = identb[:, P:].rearrange("p (a n) -> p a n", a=3)
        nc.sync.dma_start(bias_sb[:], biases[:])
        nc.sync.dma_start(identb[:], ident_in[:])
        nc.scalar.dma_start(threes_sb[:], threes[:])
        nc.sync.dma_start(whh_sb[:], whh_t[:])
        nc.sync.dma_start(wih_sb[:], wih_s[:])

        # ---- state ----
        # bf16: the accumulate then runs in the DVE's 2x mode (~560ns vs
        # ~950ns f32); the ~0.4%-per-add rounding is well inside tolerance
        hacc5 = state.tile([P, J, P5], bf16, name="hacc5", tag="hacc5")
        csum5 = state.tile([P, J, P4], f32, name="csum5", tag="csum5")
        x4 = state.tile([P, J, ARITY, P4], bf16, name="x4", tag="x4")
        # per-role gi4 tiles: level-4 step 0 needs only r/z for its id-MMs,
        # so it can start before the n-role copies land
        gi4_roles = [
            state.tile([P, 3, ARITY, P4], bf16, name=f"gi4_{r}", tag=f"gi4_{r}")
            for r in range(3)
        ]
        hacc4 = state.tile([P, J, P4], f32, name="hacc4", tag="hacc4")
        nc.vector.memset(hacc5[:], 0.0)
        nc.vector.memset(hacc4[:], 0.0)

        def rz_tile():
            # 3 PSUM banks; regions j=0..5 of NCH cols (bank j//2, half j%2)
            t_ = pspool.tile([P, J, 512], f32, name="rzps", tag="rzps", bufs=2)
            return t_, t_.rearrange("p a (b n) -> p (a b) n", b=2)

        def hn_tile():
            # 2 PSUM banks; regions m=0..2 of NCH cols in banks 0..1
            t_ = pspool.tile([P, 2, 512], f32, name="hnps", tag="hnps", bufs=1)
            return t_, t_.rearrange("p a (b n) -> p (a b) n", b=2)[:, 0:3]

        # ================= level 5: 512 parents, 2 chunks of 256 =============
        with nc.named_scope("level5"):
            h_prev = [None, None]
            for t in range(ARITY):
                h_list = []
                for ch in range(P5 // NCH):
                    sl = slice(ch * NCH, (ch + 1) * NCH)
                    gi = gi_tiles[t][ch]
                    gi_n = gi[:, 6:9, :]

                    # per-chunk h tile: contiguous reads everywhere and
                    # chunk-level readiness for the next step's matmuls.
                    # Deep ring (6): the gpsimd hacc reader lags ~5 steps
                    # behind while the standard-ucode lib loads mid-level,
                    # and h slots must not recycle through it.
                    h_new = gates.tile(
                        [P, J, NCH],
                        bf16,
                        name=f"h5_{ch}",
                        tag=f"h5_{ch}",
                        bufs=6,
                    )
                    def g5(name):
                        return gates.tile(
                            [P, J, NCH], bf16, name=name, tag=name, bufs=4
                        )

                    r_sb = g5("r5")
                    z_sb = g5("z5")
                    rhn = g5("rhn")
                    npre = g5("npre")
                    n_sb = g5("nsb")
                    t1 = g5("t1")
                    zh = g5("zh")
                    zm1 = g5("zm1")

                    if t == 0:
                        # h0 = 0: pre_rz = gi (biases folded in G); no PSUM.
                        # z_sb here holds (1-z) directly.
                        nc.scalar.activation(r_sb[:], gi[:, 0:3, :], Sig)
                        nc.scalar.activation(
                            z_sb[:], gi[:, 3:6, :], Sig, scale=-1.0
                        )
                        for m in range(J):
                            # rhn = r * b_hh_n + gi_n
                            nc.vector.scalar_tensor_tensor(
                                out=npre[:, m],
                                in0=r_sb[:, m],
                                scalar=bias_sb[:, 6 + m : 7 + m],
                                in1=gi_n[:, m],
                                op0=Mult,
                                op1=Add,
                            )
                        nc.scalar.activation(n_sb[:], npre[:], Tanh)
                        # h = (1-z)*n = w*n  (h0 = 0)
                        nc.vector.tensor_tensor(
                            out=h_new[:], in0=z_sb[:], in1=n_sb[:], op=Mult
                        )
                    else:
                        hp = h_prev[ch]
                        # rz: one 3-bank tile; per bank one full-bank id-MM
                        # (N=512, start=True) injects gi, then hh accumulates.
                        # All six rz regions fill BEFORE hn: readiness is
                        # tile-level, so sig(r) waits every rzps writer --
                        # putting hn last keeps it off sig(r)'s gate while
                        # it overlaps with sig(r) itself.
                        rzp, rzv = rz_tile()
                        for b in range(3):
                            nc.tensor.matmul(
                                rzp[:, b, :],
                                ident[:],
                                gi[:, 2 * b : 2 * b + 2, :].rearrange(
                                    "p a n -> p (a n)"
                                ),
                                start=True,
                                stop=False,
                            )
                        for j in (0, 1, 2, 3):
                            for k in range(J):
                                nc.tensor.matmul(
                                    rzv[:, j],
                                    whh_sb[:, k, j, :],
                                    hp[:, k, :],
                                    start=False,
                                    stop=(j % 2 == 1 and k == 2),
                                )
                        for j in (4, 5):
                            for k in range(J):
                                nc.tensor.matmul(
                                    rzv[:, j],
                                    whh_sb[:, k, j, :],
                                    hp[:, k, :],
                                    start=False,
                                    stop=(j == 5 and k == 2),
                                )
                        # hn: 2 banks; b_hh_n enters via identity matmuls on
                        # a broadcast-constant rhs (same ident stationary as
                        # the rz injections -- no extra LDWEIGHTS), each
                        # bank's start=True writer; hh accumulates. Keeping
                        # rhn a SINGLE vector op matters: the hn psum ring
                        # is bufs=1, so the bank is held until rhn's last
                        # read -- a spread 3-op STT here stalls the next
                        # chunk's hn matmuls on the in-order PE queue.
                        hnp, hn_v = hn_tile()
                        nc.tensor.matmul(
                            hnp[:, 0, :],
                            ident[:],
                            bvec[:, 0:2, :].rearrange("p a n -> p (a n)"),
                            start=True,
                            stop=False,
                        )
                        nc.tensor.matmul(
                            hnp[:, 1, :NCH],
                            ident[:],
                            bvec[:, 2, :],
                            start=True,
                            stop=False,
                        )
                        for m in range(J):
                            for k in range(J):
                                nc.tensor.matmul(
                                    hn_v[:, m],
                                    whh_sb[:, k, 6 + m, :],
                                    hp[:, k, :],
                                    start=False,
                                    stop=(k == 2 and m != 0),
                                )

                        nc.scalar.activation(r_sb[:], rzv[:, 0:3], Sig)
                        nc.scalar.activation(z_sb[:], rzv[:, 3:6], Sig)
                        # chain: rhn -> npre -> tanh -> t1 -> h; zh rides the
                        # DVE queue between npre and t1 (its data is ready
                        # early, and it fills the tanh window).
                        nc.vector.tensor_tensor(
                            out=rhn[:], in0=hn_v, in1=r_sb[:], op=Mult
                        )
                        nc.vector.tensor_tensor(
                            out=npre[:], in0=rhn[:], in1=gi_n, op=Add
                        )
                        nc.vector.tensor_tensor(
                            out=zh[:], in0=z_sb[:], in1=hp[:], op=Mult
                        )
                        nc.scalar.activation(zm1[:], rzv[:, 3:6], Sig, scale=-1.0)
                        nc.scalar.activation(n_sb[:], npre[:], Tanh)
                        nc.vector.tensor_tensor(
                            out=t1[:], in0=zm1[:], in1=n_sb[:], op=Mult
                        )
                        nc.vector.tensor_tensor(
                            out=h_new[:], in0=zh[:], in1=t1[:], op=Add
                        )

                    h_list.append(h_new)
                    if t == ARITY - 1:
                        qsl = slice(ch * NCH // ARITY, (ch + 1) * NCH // ARITY)
                        # child-mean of final hiddens -> h0 of level 4
                        nc.vector.tensor_reduce(
                            out=csum5[:, :, qsl],
                            in_=h_new.rearrange("p j (q c) -> p j q c", c=ARITY),
                            axis=mybir.AxisListType.X,
                            op=Add,
                        )
                        # x4 = hacc + h (raw sum; /8 folded into wih_s)
                        hperm = h_new.rearrange("p j (q c) -> p j c q", c=ARITY)
                        for j in range(J):
                            # all on vector: the gpsimd queue may still be
                            # draining its hacc backlog behind the lib swap
                            eng = nc.vector
                            eng.tensor_tensor(
                                out=x4[:, j, :, qsl],
                                in0=hacc5[:, j, sl].rearrange(
                                    "p (q c) -> p c q", c=ARITY
                                ),
                                in1=hperm[:, j],
                                op=Add,
                            )
                    else:
                        # parent-major contiguous accumulate. On the DVE,
                        # NOT gpsimd: gpsimd TTs here both queue behind the
                        # gather descgens (lib batching) and visibly slow
                        # concurrent DVE ops 2-3x while running.
                        nc.vector.tensor_tensor(
                            out=hacc5[:, :, sl],
                            in0=hacc5[:, :, sl],
                            in1=h_new[:],
                            op=Add,
                        )
                h_prev = h_list

        # ================= level 4: 64 parents, single chunk =================
        with nc.named_scope("level4"):
            h4 = gates.tile([P, J, P4], bf16, name="h4", tag="h4")
            nc.scalar.mul(h4[:], csum5[:], 1.0 / ARITY)

            # gi4 = x4 @ (w_ih/8).T + biases: one 3-bank group per role
            # (regions j' = bank, N=512 over all (child, parent) columns).
            # psum->sbuf copies split across scalar/vector so they drain in
            # ~2 copy-times, not 3 serial on the vector engine.
            # biases enter as per-partition adds on the psum->sbuf copies
            # (gi4 is feature-major, so the bias is constant per partition
            # within each m-tile) -- no bias matmuls on the PE.
            bias_col = {0: 0, 1: 3, 2: 9}
            for role in range(3):
                rzp, _ = rz_tile()
                for jj in range(3):
                    for k in range(J):
                        nc.tensor.matmul(
                            rzp[:, jj, :],
                            wih_sb[:, k, 3 * role + jj, :],
                            x4[:, k].rearrange("p c q -> p (c q)"),
                            start=(k == 0),
                            stop=(k == 2),
                        )
                for m in range(3):
                    src = rzp[:, m, :].rearrange("p (c q) -> p c q", c=ARITY)
                    dst = gi4_roles[role][:, m]
                    bcol = bias_col[role] + m
                    if (role + m) % 2 == 0:
                        nc.scalar.add(
                            out=dst, in_=src, add=bias_sb[:, bcol : bcol + 1]
                        )
                    else:
                        nc.vector.tensor_scalar_add(
                            out=dst,
                            in0=src,
                            scalar1=bias_sb[:, bcol : bcol + 1],
                        )

            for t in range(ARITY):
                c = ARITY - 1 - t
                gi_n = gi4_roles[2][:, :, c, :]

                r_sb = gates.tile([P, J, P4], bf16, name="r4", tag="r4")
                z_sb = gates.tile([P, J, P4], bf16, name="z4", tag="z4")
                rhn = gates.tile([P, J, P4], bf16, name="rhn4", tag="rhn4")
                npre = gates.tile([P, J, P4], bf16, name="npre4", tag="npre4")
                n_sb = gates.tile([P, J, P4], bf16, name="nsb4", tag="nsb4")
                t1 = gates.tile([P, J, P4], bf16, name="t14", tag="t14")
                zh4 = gates.tile([P, J, P4], bf16, name="zh4", tag="zh4")
                zm14 = gates.tile([P, J, P4], bf16, name="zm14", tag="zm14")
                h4n = gates.tile([P, J, P4], bf16, name="h4", tag="h4")

                # r, hn, and z each fill their OWN psum tile instance so
                # readers wait only their own writers: sig(r) fires after the
                # 10 r-side MMs, not the whole 30-MM burst. id-MMs first
                # (h-independent), then hh r, hn, hh z.
                rzpA, _ = rz_tile()
                rv = rzpA[:, 0, :].rearrange("p (m n) -> p m n", m=8)[:, 0:3]
                nc.tensor.matmul(
                    rzpA[:, 0, : 3 * P4].rearrange("p (a n) -> p a n", a=3),
                    ident[:],
                    gi4_roles[0][:, :, c, :],
                    start=True,
                    stop=False,
                )
                for m in range(J):
                    for k in range(J):
                        nc.tensor.matmul(
                            rv[:, m],
                            whh_sb[:, k, m, :],
                            h4[:, k, :],
                            start=False,
                            stop=(m == 2 and k == 2),
                        )
                hnp, _ = hn_tile()
                hn_v = hnp[:, 0, :].rearrange("p (m n) -> p m n", m=8)[:, 0:3]
                nc.tensor.matmul(
                    hnp[:, 0, : 3 * P4],
                    biasmm_sb[:, 3, :],
                    onehot4_sb[:, :, :].rearrange("k m n -> k (m n)"),
                    start=True,
                    stop=False,
                )
                for m in range(J):
                    for k in range(J):
                        nc.tensor.matmul(
                            hn_v[:, m],
                            whh_sb[:, k, 6 + m, :],
                            h4[:, k, :],
                            start=False,
                            stop=(m == 2 and k == 2),
                        )
                rzpB, _ = rz_tile()
                zv = rzpB[:, 0, :].rearrange("p (m n) -> p m n", m=8)[:, 0:3]
                nc.tensor.matmul(
                    rzpB[:, 0, : 3 * P4].rearrange("p (a n) -> p a n", a=3),
                    ident[:],
                    gi4_roles[1][:, :, c, :],
                    start=True,
                    stop=False,
                )
                for m in range(J):
                    for k in range(J):
                        nc.tensor.matmul(
                            zv[:, m],
                            whh_sb[:, k, 3 + m, :],
                            h4[:, k, :],
                            start=False,
                            stop=(m == 2 and k == 2),
                        )

                nc.scalar.activation(r_sb[:], rv, Sig)
                nc.scalar.activation(z_sb[:], zv, Sig)
                nc.vector.tensor_tensor(out=rhn[:], in0=hn_v, in1=r_sb[:], op=Mult)
                nc.vector.tensor_tensor(out=npre[:], in0=rhn[:], in1=gi_n, op=Add)
                nc.vector.tensor_tensor(out=zh4[:], in0=z_sb[:], in1=h4[:], op=Mult)
                nc.scalar.activation(zm14[:], zv, Sig, scale=-1.0)
                nc.scalar.activation(n_sb[:], npre[:], Tanh)
                nc.vector.tensor_tensor(out=t1[:], in0=zm14[:], in1=n_sb[:], op=Mult)
                nc.vector.tensor_tensor(out=h4n[:], in0=zh4[:], in1=t1[:], op=Add)
                # hacc runs off-chain on the (idle) gpsimd engine
                nc.gpsimd.tensor_tensor(
                    out=hacc4[:], in0=hacc4[:], in1=h4n[:], op=Add
                )
                h4 = h4n

            # ---- ship level-4 outputs ----
            nc.sync.dma_start(out_x[:], hacc4[:])
            nc.sync.dma_start(out_h[:], h4[:])


def _build_program():
    if "nc" in _PROG_CACHE:
        return _PROG_CACHE["nc"]
    import concourse.bacc as bacc
    import concourse.mybir as mybir
    import concourse.tile as tile

    f32 = mybir.dt.float32
    bf16 = mybir.dt.bfloat16

    nc = bacc.Bacc(
        "TRN2",
        target_bir_lowering=False,
        debug=False,
        enable_asserts=False,
        num_devices=NCORES,
        num_swdge_queues=4,
    )
    gtab = nc.dram_tensor("gtab", [VOCAB, 9 * P], bf16, kind="ExternalInput").ap()
    idxs = nc.dram_tensor(
        "idxs", [P, 2 * ARITY, NCH // 16], mybir.dt.int16, kind="ExternalInput"
    ).ap()
    ident_in = nc.dram_tensor(
        "ident", [P, P + 3 * NCH], bf16, kind="ExternalInput"
    ).ap()
    whh_t = nc.dram_tensor("whh_t", [P, J, 9, P], bf16, kind="ExternalInput").ap()
    wih_s = nc.dram_tensor("wih_s", [P, J, 9, P], bf16, kind="ExternalInput").ap()
    biases = nc.dram_tensor("biases", [P, 12], f32, kind="ExternalInput").ap()
    threes = nc.dram_tensor("threes", [3, 3008], bf16, kind="ExternalInput").ap()
    out_x = nc.dram_tensor("out_x", [P, J, P4], f32, kind="ExternalOutput").ap()
    out_h = nc.dram_tensor("out_h", [P, J, P4], bf16, kind="ExternalOutput").ap()

    with tile.TileContext(nc) as tc:
        _emit(
            tc,
            nc,
            (gtab, idxs, ident_in, whh_t, wih_s, biases, threes, out_x, out_h),
        )
    nc.compile()
    _PROG_CACHE["nc"] = nc
    return nc


def _retile_weights(w):
    # w: [1152, 384] -> lhsT tiles [128(k_part), 3(k), 9(m), 128(m_col)] bf16
    wt = np.ascontiguousarray(w.T)  # [384, 1152]
    wt = wt.reshape(J, P, 9, P).transpose(1, 0, 2, 3)
    return np.ascontiguousarray(wt).astype(BF16)


def _prep_bias(b_ih, b_hh):
    biases = np.zeros((P, 12), np.float32)
    comb = (b_ih + b_hh).reshape(9, P)
    biases[:, 0:6] = comb[0:6].T
    biases[:, 6:9] = b_hh.reshape(9, P)[6:9].T
    biases[:, 9:12] = b_ih.reshape(9, P)[6:9].T
    return biases


def _prep_biasmm(b_ih, b_hh):
    # biasmm[k, role, p]: roles 0..2 feed the gi4 bias matmuls (combined r/z
    # bias and b_ih_n); role 3 is b_hh_n for the hn-bank bias matmuls.
    bias12 = np.zeros((P, 12), np.float32)
    comb = (b_ih + b_hh).reshape(9, P)
    bias12[:, 0:6] = comb[0:6].T
    bias12[:, 6:9] = b_ih.reshape(9, P)[6:9].T
    bias12[:, 9:12] = b_hh.reshape(9, P)[6:9].T
    out = bias12.T.reshape(4, 3, P).transpose(1, 0, 2)
    return np.ascontiguousarray(out).astype(BF16)


def _prep_onehot(n):
    out = np.zeros((3, 3, n), np.float32)
    for k in range(3):
        out[k, k, :] = 1.0
    return out.astype(BF16)


def _prep_idxs(tokens_core):
    # child-major half-gathers: gather (g, h) covers child 7-g, parents
    # [256h, 256h+256), wrapped [16, 16] (idx i -> [i%16, i//16]) and
    # replicated over the 8 Q7 stripes.
    byc = tokens_core.reshape(P5, ARITY)  # [parent, child]
    out = np.zeros((P, 2 * ARITY, NCH // 16), np.int16)
    for g in range(ARITY):
        c = ARITY - 1 - g
        for h in range(2):
            seg = byc[h * NCH : (h + 1) * NCH, c]
            wrapped = seg.reshape(NCH // 16, 16).T.astype(np.int16)
            out[:, 2 * g + h, :] = np.tile(wrapped, (8, 1))
    return out


def _gru_step_batch(x_t, h, w_ih, w_hh, b_ih, b_hh):
    gi = x_t @ w_ih.T + b_ih
    gh = h @ w_hh.T + b_hh
    i_r, i_z, i_n = np.split(gi, 3, axis=-1)
    h_r, h_z, h_n = np.split(gh, 3, axis=-1)
    r = 1.0 / (1.0 + np.exp(-(i_r + h_r)))
    z = 1.0 / (1.0 + np.exp(-(i_z + h_z)))
    n = np.tanh(i_n + r * h_n)
    return (1.0 - z) * n + z * h


def _host_level(xs, h0, w_ih, w_hh, b_ih, b_hh):
    # xs: [n_parents, ARITY, D] child outputs in natural child order.
    h = h0
    acc = np.zeros_like(h)
    for t in range(ARITY):
        h = _gru_step_batch(xs[:, ARITY - 1 - t], h, w_ih, w_hh, b_ih, b_hh)
        acc += h
    return acc / ARITY, h


def kernel(leaf_tokens, embed_table, w_ih, w_hh, b_ih, b_hh):
    from concourse.bass_utils import run_bass_kernel_spmd

    leaf_tokens = np.asarray(leaf_tokens, np.int32)
    embed_table = np.asarray(embed_table, np.float32)
    w_ih = np.asarray(w_ih, np.float32)
    w_hh = np.asarray(w_hh, np.float32)
    b_ih = np.asarray(b_ih, np.float32)
    b_hh = np.asarray(b_hh, np.float32)

    nc = _build_program()

    # G = embed @ w_ih.T with r/z biases (b_ih+b_hh) and n bias (b_ih) folded.
    bias_fold = np.concatenate([(b_ih + b_hh)[: 2 * DIM], b_ih[2 * DIM :]])
    G = embed_table @ w_ih.T
    G += bias_fold
    G = G.astype(BF16)

    whh_t = _retile_weights(w_hh)
    wih_s = _retile_weights(w_ih / ARITY)
    biases = _prep_bias(b_ih, b_hh)
    identity = np.zeros((P, P + 3 * NCH), np.float32)
    identity[:, 0:P] = np.eye(P, dtype=np.float32)
    bhn = b_hh[2 * DIM :].reshape(3, P)  # [m, p]
    identity[:, P:] = np.repeat(bhn.T[:, :, None], NCH, axis=2).reshape(P, 3 * NCH)
    identity = identity.astype(BF16)
    threes = np.zeros((3, 3008), BF16)
    threes[:, 0:512] = _prep_biasmm(b_ih, b_hh).reshape(3, 512)
    threes[:, 512:1280] = _prep_onehot(NCH).reshape(3, 768)
    threes[:, 1280:2816] = _prep_onehot(512).reshape(3, 1536)
    threes[:, 2816:3008] = _prep_onehot(P4).reshape(3, 192)

    in_maps = []
    for core in range(NCORES):
        toks = leaf_tokens[core * LEAVES_CORE : (core + 1) * LEAVES_CORE]
        in_maps.append(
            {
                "gtab": G,
                "idxs": _prep_idxs(toks),
                "ident": identity,
                "whh_t": whh_t,
                "wih_s": wih_s,
                "biases": biases,
                "threes": threes,
            }
        )
    res = run_bass_kernel_spmd(nc, in_maps, core_ids=list(range(NCORES)))

    # ---- host epilogue: levels 3, 2 (per core) and the root ----
    w_ih64 = w_ih.astype(np.float64)
    w_hh64 = w_hh.astype(np.float64)
    b_ih64 = b_ih.astype(np.float64)
    b_hh64 = b_hh.astype(np.float64)

    x3 = np.zeros((NCORES, P4, DIM), np.float64)
    h4 = np.zeros((NCORES, P4, DIM), np.float64)
    for core in range(NCORES):
        hacc = np.asarray(res.results[core]["out_x"], np.float64)  # [128,3,64]
        hh = np.asarray(
            res.results[core]["out_h"].astype(np.float32), np.float64
        )  # [128,3,64]
        # [p, k, node] -> node-major [node, feat=128k+p]
        x3[core] = hacc.transpose(2, 1, 0).reshape(P4, DIM) / ARITY
        h4[core] = hh.transpose(2, 1, 0).reshape(P4, DIM)

    # level 3: 8 parents per core
    xs3 = x3.reshape(NCORES * ARITY, ARITY, DIM)
    h03 = h4.reshape(NCORES * ARITY, ARITY, DIM).mean(axis=1)
    x2, h3 = _host_level(xs3, h03, w_ih64, w_hh64, b_ih64, b_hh64)
    # level 2: 1 parent per core
    xs2 = x2.reshape(NCORES, ARITY, DIM)
    h02 = h3.reshape(NCORES, ARITY, DIM).mean(axis=1)
    x1, h2 = _host_level(xs2, h02, w_ih64, w_hh64, b_ih64, b_hh64)
    # root: 1 node over the 8 cores' outputs
    xs1 = x1.reshape(1, ARITY, DIM)
    h01 = h2.reshape(1, ARITY, DIM).mean(axis=1)
    x0, _ = _host_level(xs1, h01, w_ih64, w_hh64, b_ih64, b_hh64)

    return x0.astype(np.float32).reshape(1, 1, DIM)



# revision 40
# speedup vs baseline: 1.1888x; 1.1888x over previous
"""Tree-GRU (arity-8, depth-5) over embedded leaves on 8 TRN2 NeuronCores.

Sharding: data-parallel over subtrees. Each core takes 4096 contiguous leaves
and runs the two large tree levels locally (512 -> 64 parents). The three tiny
tail levels (8 -> 1 parents per core, plus the root across cores) are
latency-bound chains of 8-wide GRU steps over <=73 nodes total; they run on
host from the shipped level-4 outputs (the sharding hint's "all-gather the
last log2(M) levels" relaxation).

The leaf level never materializes embeddings on device: the input-gate
projection G = embed_table @ w_ih.T (+ folded biases) is precomputed on host
(token-independent weight prep, like the weight retiling) and the kernel
gathers G rows per leaf with dma_gather(transpose=True), which lands the rows
directly in feature-major layout [128, 9, leaves] -- no PE transposes and no
on-device input-gate matmuls for the leaf level. Tokens are pre-permuted
child-major so each gather covers exactly one (step, chunk)'s children;
gathers fan out over 4 SWDGE queues and the gpsimd queue carries nothing else
until the last gather issues.

Per step the r/z pre-activations accumulate in one 3-bank PSUM tile: gathered
gi tiles are injected with identity matmuls (region-ordered start=True bank
clears), recurrent h @ w_hh.T tiles accumulate on top, and a single sigmoid
reads the whole tile. The n-role pre-activation gets b_hh_n from a one-hot
bias matmul (the hn bank's start=True writer), so the gate chain is one
bf16 multiply and one bf16 add before the tanh. Biases ride in G (b_ih +
b_hh for r/z, b_ih for n). Hidden states live in a 3-deep ring of per-step
tiles and both hidden accumulators are parent-major, so the (slow,
late-starting) gpsimd accumulator adds run contiguous and off the chain.
"""

import numpy as np
import ml_dtypes

ARITY = 8
DIM = 384
VOCAB = 32000
NCORES = 8
P = 128
J = 3  # DIM // 128 feature tiles
N_LEAVES = 32768
LEAVES_CORE = N_LEAVES // NCORES  # 4096
P5 = LEAVES_CORE // ARITY  # 512 level-5 parents per core
P4 = P5 // ARITY  # 64 level-4 parents per core
NCH = 256  # level-5 chunk size (2 chunks)

BF16 = ml_dtypes.bfloat16

_PROG_CACHE = {}


def _emit(tc, nc, aps):
    import concourse.mybir as mybir

    f32 = mybir.dt.float32
    bf16 = mybir.dt.bfloat16
    i16 = mybir.dt.int16
    Sig = mybir.ActivationFunctionType.Sigmoid
    Tanh = mybir.ActivationFunctionType.Tanh
    Add = mybir.AluOpType.add
    Sub = mybir.AluOpType.subtract
    Mult = mybir.AluOpType.mult

    (gtab, idxs, ident_in, whh_t, wih_s, biases, threes, out_x, out_h) = aps

    from contextlib import ExitStack

    with ExitStack() as ctx:
        const = ctx.enter_context(tc.tile_pool(name="const", bufs=1))
        gpool = ctx.enter_context(tc.tile_pool(name="gpool", bufs=1))
        state = ctx.enter_context(tc.tile_pool(name="state", bufs=1))
        gates = ctx.enter_context(tc.tile_pool(name="gates", bufs=3))
        pspool = ctx.enter_context(tc.tile_pool(name="pspool", bufs=1, space="PSUM"))

        # ---- prologue: idxs first, then the 16 transposing half-gathers ----
        # ONE idx DMA: with a single completion semaphore the first gather's
        # wait stays inline on the gather op, so the auto-inserted gather-lib
        # load (~13us ucode fetch) runs ahead of the wait instead of behind
        # an idx-wait hoisted in front of it.
        idx_sb = const.tile([P, 2 * ARITY, NCH // 16], i16)
        nc.scalar.dma_start(idx_sb[:], idxs[:])

        gi_tiles = []  # [t][ch] -> [128, 9, 256]
        for g in range(ARITY):
            halves = []
            for h in range(2):
                gi = gpool.tile([P, 9, NCH], bf16, name=f"gi{g}_{h}", tag=f"gi{g}_{h}")
                nc.gpsimd.dma_gather(
                    out_ap=gi[:],
                    in_ap=gtab[:],
                    idxs_ap=idx_sb[:, 2 * g + h, :],
                    num_idxs=NCH,
                    num_idxs_reg=NCH,
                    elem_size=9 * P,
                    transpose=True,
                    queue_num=(2 * g + h) % 4,
                )
                halves.append(gi)
            gi_tiles.append(halves)

        # ---- constants / weights (overlap with gathers) ----
        # 7 DMAs total (2 idx + 5 here) so no completion-semaphore reuse.
        # The [3, x] matmul constants ride in one packed blob.
        whh_sb = const.tile([P, J, 9, P], bf16)
        wih_sb = const.tile([P, J, 9, P], bf16)
        bias_sb = const.tile([P, 12], f32)
        threes_sb = const.tile([3, 3008], bf16)
        biasmm_sb = threes_sb[:, 0:512].rearrange("p (a n) -> p a n", a=4)
        onehot_sb = threes_sb[:, 512:1280].rearrange("p (a n) -> p a n", a=3)
        onehot512_sb = threes_sb[:, 1280:2816].rearrange("p (a n) -> p a n", a=3)
        onehot4_sb = threes_sb[:, 2816:3008].rearrange("p (a n) -> p a n", a=3)
        # ident blob: [:, 0:128] identity, [:, 128:896] b_hh_n broadcast over
        # NCH cols per m-tile (rhs of the hn bias id-MMs -- reuses the
        # already-loaded identity stationary, no biasmm LDWEIGHTS)
        identb = const.tile([P, P + 3 * NCH], bf16)
        ident = identb[:, 0:P]
        bvec = identb[:, P:].rearrange("p (a n) -> p a n", a=3)
        nc.sync.dma_start(bias_sb[:], biases[:])
        nc.sync.dma_start(identb[:], ident_in[:])
        nc.scalar.dma_start(threes_sb[:], threes[:])
        nc.sync.dma_start(whh_sb[:], whh_t[:])
        nc.sync.dma_start(wih_sb[:], wih_s[:])

        # ---- state ----
        # bf16: the accumulate then runs in the DVE's 2x mode (~560ns vs
        # ~950ns f32); the ~0.4%-per-add rounding is well inside tolerance
        hacc5 = state.tile([P, J, P5], bf16, name="hacc5", tag="hacc5")
        csum5 = state.tile([P, J, P4], f32, name="csum5", tag="csum5")
        x4 = state.tile([P, J, ARITY, P4], bf16, name="x4", tag="x4")
        # per-role gi4 tiles: level-4 step 0 needs only r/z for its id-MMs,
        # so it can start before the n-role copies land
        gi4_roles = [
            state.tile([P, 3, ARITY, P4], bf16, name=f"gi4_{r}", tag=f"gi4_{r}")
            for r in range(3)
        ]
        hacc4 = state.tile([P, J, P4], f32, name="hacc4", tag="hacc4")
        nc.vector.memset(hacc5[:], 0.0)
        nc.vector.memset(hacc4[:], 0.0)

        def rz_tile():
            # 3 PSUM banks; regions j=0..5 of NCH cols (bank j//2, half j%2)
            t_ = pspool.tile([P, J, 512], f32, name="rzps", tag="rzps", bufs=2)
            return t_, t_.rearrange("p a (b n) -> p (a b) n", b=2)

        def hn_tile():
            # 2 PSUM banks; regions m=0..2 of NCH cols in banks 0..1
            t_ = pspool.tile([P, 2, 512], f32, name="hnps", tag="hnps", bufs=1)
            return t_, t_.rearrange("p a (b n) -> p (a b) n", b=2)[:, 0:3]

        # ================= level 5: 512 parents, 2 chunks of 256 =============
        with nc.named_scope("level5"):
            h_prev = [None, None]
            for t in range(ARITY):
                h_list = []
                for ch in range(P5 // NCH):
                    sl = slice(ch * NCH, (ch + 1) * NCH)
                    gi = gi_tiles[t][ch]
                    gi_n = gi[:, 6:9, :]

                    # per-chunk h tile: contiguous reads everywhere and
                    # chunk-level readiness for the next step's matmuls.
                    # Deep ring (6): the gpsimd hacc reader lags ~5 steps
                    # behind while the standard-ucode lib loads mid-level,
                    # and h slots must not recycle through it.
                    h_new = gates.tile(
                        [P, J, NCH],
                        bf16,
                        name=f"h5_{ch}",
                        tag=f"h5_{ch}",
                        bufs=6,
                    )
                    def g5(name):
                        return gates.tile(
                            [P, J, NCH], bf16, name=name, tag=name
                        )

                    r_sb = g5("r5")
                    z_sb = g5("z5")
                    rhn = g5("rhn")
                    npre = g5("npre")
                    n_sb = g5("nsb")
                    t1 = g5("t1")
                    zh = g5("zh")
                    zm1 = g5("zm1")

                    if t == 0:
                        # h0 = 0: pre_rz = gi (biases folded in G); no PSUM.
                        # z_sb here holds (1-z) directly.
                        nc.scalar.activation(r_sb[:], gi[:, 0:3, :], Sig)
                        nc.scalar.activation(
                            z_sb[:], gi[:, 3:6, :], Sig, scale=-1.0
                        )
                        for m in range(J):
                            # rhn = r * b_hh_n + gi_n
                            nc.vector.scalar_tensor_tensor(
                                out=npre[:, m],
                                in0=r_sb[:, m],
                                scalar=bias_sb[:, 6 + m : 7 + m],
                                in1=gi_n[:, m],
                                op0=Mult,
                                op1=Add,
                            )
                        nc.scalar.activation(n_sb[:], npre[:], Tanh)
                        # h = (1-z)*n = w*n  (h0 = 0)
                        nc.vector.tensor_tensor(
                            out=h_new[:], in0=z_sb[:], in1=n_sb[:], op=Mult
                        )
                    else:
                        hp = h_prev[ch]
                        # rz: one 3-bank tile; per bank one full-bank id-MM
                        # (N=512, start=True) injects gi, then hh accumulates.
                        # All six rz regions fill BEFORE hn: readiness is
                        # tile-level, so sig(r) waits every rzps writer --
                        # putting hn last keeps it off sig(r)'s gate while
                        # it overlaps with sig(r) itself.
                        rzp, rzv = rz_tile()
                        for b in range(3):
                            nc.tensor.matmul(
                                rzp[:, b, :],
                                ident[:],
                                gi[:, 2 * b : 2 * b + 2, :].rearrange(
                                    "p a n -> p (a n)"
                                ),
                                start=True,
                                stop=False,
                            )
                        for j in (0, 1, 2, 3):
                            for k in range(J):
                                nc.tensor.matmul(
                                    rzv[:, j],
                                    whh_sb[:, k, j, :],
                                    hp[:, k, :],
                                    start=False,
                                    stop=(j % 2 == 1 and k == 2),
                                )
                        for j in (4, 5):
                            for k in range(J):
                                nc.tensor.matmul(
                                    rzv[:, j],
                                    whh_sb[:, k, j, :],
                                    hp[:, k, :],
                                    start=False,
                                    stop=(j == 5 and k == 2),
                                )
                        # hn: 2 banks; b_hh_n enters via identity matmuls on
                        # a broadcast-constant rhs (same ident stationary as
                        # the rz injections -- no extra LDWEIGHTS), each
                        # bank's start=True writer; hh accumulates. Keeping
                        # rhn a SINGLE vector op matters: the hn psum ring
                        # is bufs=1, so the bank is held until rhn's last
                        # read -- a spread 3-op STT here stalls the next
                        # chunk's hn matmuls on the in-order PE queue.
                        hnp, hn_v = hn_tile()
                        nc.tensor.matmul(
                            hnp[:, 0, :],
                            ident[:],
                            bvec[:, 0:2, :].rearrange("p a n -> p (a n)"),
                            start=True,
                            stop=False,
                        )
                        nc.tensor.matmul(
                            hnp[:, 1, :NCH],
                            ident[:],
                            bvec[:, 2, :],
                            start=True,
                            stop=False,
                        )
                        for m in range(J):
                            for k in range(J):
                                nc.tensor.matmul(
                                    hn_v[:, m],
                                    whh_sb[:, k, 6 + m, :],
                                    hp[:, k, :],
                                    start=False,
                                    stop=(k == 2 and m != 0),
                                )

                        nc.scalar.activation(r_sb[:], rzv[:, 0:3], Sig)
                        nc.scalar.activation(z_sb[:], rzv[:, 3:6], Sig)
                        # chain: rhn -> npre -> tanh -> t1 -> h; zh rides the
                        # DVE queue between npre and t1 (its data is ready
                        # early, and it fills the tanh window).
                        nc.vector.tensor_tensor(
                            out=rhn[:], in0=hn_v, in1=r_sb[:], op=Mult
                        )
                        nc.vector.tensor_tensor(
                            out=npre[:], in0=rhn[:], in1=gi_n, op=Add
                        )
                        nc.vector.tensor_tensor(
                            out=zh[:], in0=z_sb[:], in1=hp[:], op=Mult
                        )
                        nc.scalar.activation(zm1[:], rzv[:, 3:6], Sig, scale=-1.0)
                        nc.scalar.activation(n_sb[:], npre[:], Tanh)
                        nc.vector.tensor_tensor(
                            out=t1[:], in0=zm1[:], in1=n_sb[:], op=Mult
                        )
                        nc.vector.tensor_tensor(
                            out=h_new[:], in0=zh[:], in1=t1[:], op=Add
                        )

                    h_list.append(h_new)
                    if t == ARITY - 1:
                        qsl = slice(ch * NCH // ARITY, (ch + 1) * NCH // ARITY)
                        # child-mean of final hiddens -> h0 of level 4
                        nc.vector.tensor_reduce(
                            out=csum5[:, :, qsl],
                            in_=h_new.rearrange("p j (q c) -> p j q c", c=ARITY),
                            axis=mybir.AxisListType.X,
                            op=Add,
                        )
                        # x4 = hacc + h (raw sum; /8 folded into wih_s)
                        hperm = h_new.rearrange("p j (q c) -> p j c q", c=ARITY)
                        for j in range(J):
                            # all on vector: the gpsimd queue may still be
                            # draining its hacc backlog behind the lib swap
                            eng = nc.vector
                            eng.tensor_tensor(
                                out=x4[:, j, :, qsl],
                                in0=hacc5[:, j, sl].rearrange(
                                    "p (q c) -> p c q", c=ARITY
                                ),
                                in1=hperm[:, j],
                                op=Add,
                            )
                    else:
                        # parent-major contiguous accumulate. On the DVE,
                        # NOT gpsimd: gpsimd TTs here both queue behind the
                        # gather descgens (lib batching) and visibly slow
                        # concurrent DVE ops 2-3x while running.
                        nc.vector.tensor_tensor(
                            out=hacc5[:, :, sl],
                            in0=hacc5[:, :, sl],
                            in1=h_new[:],
                            op=Add,
                        )
                h_prev = h_list

        # ================= level 4: 64 parents, single chunk =================
        with nc.named_scope("level4"):
            h4 = gates.tile([P, J, P4], bf16, name="h4", tag="h4")
            nc.scalar.mul(h4[:], csum5[:], 1.0 / ARITY)

            # gi4 = x4 @ (w_ih/8).T + biases: one 3-bank group per role
            # (regions j' = bank, N=512 over all (child, parent) columns).
            # psum->sbuf copies split across scalar/vector so they drain in
            # ~2 copy-times, not 3 serial on the vector engine.
            # biases enter as per-partition adds on the psum->sbuf copies
            # (gi4 is feature-major, so the bias is constant per partition
            # within each m-tile) -- no bias matmuls on the PE.
            bias_col = {0: 0, 1: 3, 2: 9}
            for role in range(3):
                rzp, _ = rz_tile()
                for jj in range(3):
                    for k in range(J):
                        nc.tensor.matmul(
                            rzp[:, jj, :],
                            wih_sb[:, k, 3 * role + jj, :],
                            x4[:, k].rearrange("p c q -> p (c q)"),
                            start=(k == 0),
                            stop=(k == 2),
                        )
                for m in range(3):
                    src = rzp[:, m, :].rearrange("p (c q) -> p c q", c=ARITY)
                    dst = gi4_roles[role][:, m]
                    bcol = bias_col[role] + m
                    if (role + m) % 2 == 0:
                        nc.scalar.add(
                            out=dst, in_=src, add=bias_sb[:, bcol : bcol + 1]
                        )
                    else:
                        nc.vector.tensor_scalar_add(
                            out=dst,
                            in0=src,
                            scalar1=bias_sb[:, bcol : bcol + 1],
                        )

            for t in range(ARITY):
                c = ARITY - 1 - t
                gi_n = gi4_roles[2][:, :, c, :]

                r_sb = gates.tile([P, J, P4], bf16, name="r4", tag="r4")
                z_sb = gates.tile([P, J, P4], bf16, name="z4", tag="z4")
                rhn = gates.tile([P, J, P4], bf16, name="rhn4", tag="rhn4")
                npre = gates.tile([P, J, P4], bf16, name="npre4", tag="npre4")
                n_sb = gates.tile([P, J, P4], bf16, name="nsb4", tag="nsb4")
                t1 = gates.tile([P, J, P4], bf16, name="t14", tag="t14")
                zh4 = gates.tile([P, J, P4], bf16, name="zh4", tag="zh4")
                zm14 = gates.tile([P, J, P4], bf16, name="zm14", tag="zm14")
                h4n = gates.tile([P, J, P4], bf16, name="h4", tag="h4")

                # r, hn, and z each fill their OWN psum tile instance so
                # readers wait only their own writers: sig(r) fires after the
                # 10 r-side MMs, not the whole 30-MM burst. id-MMs first
                # (h-independent), then hh r, hn, hh z.
                rzpA, _ = rz_tile()
                rv = rzpA[:, 0, :].rearrange("p (m n) -> p m n", m=8)[:, 0:3]
                nc.tensor.matmul(
                    rzpA[:, 0, : 3 * P4].rearrange("p (a n) -> p a n", a=3),
                    ident[:],
                    gi4_roles[0][:, :, c, :],
                    start=True,
                    stop=False,
                )
                for m in range(J):
                    for k in range(J):
                        nc.tensor.matmul(
                            rv[:, m],
                            whh_sb[:, k, m, :],
                            h4[:, k, :],
                            start=False,
                            stop=(m == 2 and k == 2),
                        )
                hnp, _ = hn_tile()
                hn_v = hnp[:, 0, :].rearrange("p (m n) -> p m n", m=8)[:, 0:3]
                nc.tensor.matmul(
                    hnp[:, 0, : 3 * P4],
                    biasmm_sb[:, 3, :],
                    onehot4_sb[:, :, :].rearrange("k m n -> k (m n)"),
                    start=True,
                    stop=False,
                )
                for m in range(J):
                    for k in range(J):
                        nc.tensor.matmul(
                            hn_v[:, m],
                            whh_sb[:, k, 6 + m, :],
                            h4[:, k, :],
                            start=False,
                            stop=(m == 2 and k == 2),
                        )
                rzpB, _ = rz_tile()
                zv = rzpB[:, 0, :].rearrange("p (m n) -> p m n", m=8)[:, 0:3]
                nc.tensor.matmul(
                    rzpB[:, 0, : 3 * P4].rearrange("p (a n) -> p a n", a=3),
                    ident[:],
                    gi4_roles[1][:, :, c, :],
                    start=True,
                    stop=False,
                )
                for m in range(J):
                    for k in range(J):
                        nc.tensor.matmul(
                            zv[:, m],
                            whh_sb[:, k, 3 + m, :],
                            h4[:, k, :],
                            start=False,
                            stop=(m == 2 and k == 2),
                        )

                nc.scalar.activation(r_sb[:], rv, Sig)
                nc.scalar.activation(z_sb[:], zv, Sig)
                nc.vector.tensor_tensor(out=rhn[:], in0=hn_v, in1=r_sb[:], op=Mult)
                nc.vector.tensor_tensor(out=npre[:], in0=rhn[:], in1=gi_n, op=Add)
                nc.vector.tensor_tensor(out=zh4[:], in0=z_sb[:], in1=h4[:], op=Mult)
                nc.scalar.activation(zm14[:], zv, Sig, scale=-1.0)
                nc.scalar.activation(n_sb[:], npre[:], Tanh)
                nc.vector.tensor_tensor(out=t1[:], in0=zm14[:], in1=n_sb[:], op=Mult)
                nc.vector.tensor_tensor(out=h4n[:], in0=zh4[:], in1=t1[:], op=Add)
                # hacc runs off-chain on the (idle) gpsimd engine
                nc.gpsimd.tensor_tensor(
                    out=hacc4[:], in0=hacc4[:], in1=h4n[:], op=Add
                )
                h4 = h4n

            # ---- ship level-4 outputs ----
            nc.sync.dma_start(out_x[:], hacc4[:])
            nc.sync.dma_start(out_h[:], h4[:])


def _build_program():
    if "nc" in _PROG_CACHE:
        return _PROG_CACHE["nc"]
    import concourse.bacc as bacc
    import concourse.mybir as mybir
    import concourse.tile as tile

    f32 = mybir.dt.float32
    bf16 = mybir.dt.bfloat16

    nc = bacc.Bacc(
        "TRN2",
        target_bir_lowering=False,
        debug=False,
        enable_asserts=False,
        num_devices=NCORES,
        num_swdge_queues=4,
    )
    gtab = nc.dram_tensor("gtab", [VOCAB, 9 * P], bf16, kind="ExternalInput").ap()
    idxs = nc.dram_tensor(
        "idxs", [P, 2 * ARITY, NCH // 16], mybir.dt.int16, kind="ExternalInput"
    ).ap()
    ident_in = nc.dram_tensor(
        "ident", [P, P + 3 * NCH], bf16, kind="ExternalInput"
    ).ap()
    whh_t = nc.dram_tensor("whh_t", [P, J, 9, P], bf16, kind="ExternalInput").ap()
    wih_s = nc.dram_tensor("wih_s", [P, J, 9, P], bf16, kind="ExternalInput").ap()
    biases = nc.dram_tensor("biases", [P, 12], f32, kind="ExternalInput").ap()
    threes = nc.dram_tensor("threes", [3, 3008], bf16, kind="ExternalInput").ap()
    out_x = nc.dram_tensor("out_x", [P, J, P4], f32, kind="ExternalOutput").ap()
    out_h = nc.dram_tensor("out_h", [P, J, P4], bf16, kind="ExternalOutput").ap()

    with tile.TileContext(nc) as tc:
        _emit(
            tc,
            nc,
            (gtab, idxs, ident_in, whh_t, wih_s, biases, threes, out_x, out_h),
        )
    nc.compile()
    _PROG_CACHE["nc"] = nc
    return nc


def _retile_weights(w):
    # w: [1152, 384] -> lhsT tiles [128(k_part), 3(k), 9(m), 128(m_col)] bf16
    wt = np.ascontiguousarray(w.T)  # [384, 1152]
    wt = wt.reshape(J, P, 9, P).transpose(1, 0, 2, 3)
    return np.ascontiguousarray(wt).astype(BF16)


def _prep_bias(b_ih, b_hh):
    biases = np.zeros((P, 12), np.float32)
    comb = (b_ih + b_hh).reshape(9, P)
    biases[:, 0:6] = comb[0:6].T
    biases[:, 6:9] = b_hh.reshape(9, P)[6:9].T
    biases[:, 9:12] = b_ih.reshape(9, P)[6:9].T
    return biases


def _prep_biasmm(b_ih, b_hh):
    # biasmm[k, role, p]: roles 0..2 feed the gi4 bias matmuls (combined r/z
    # bias and b_ih_n); role 3 is b_hh_n for the hn-bank bias matmuls.
    bias12 = np.zeros((P, 12), np.float32)
    comb = (b_ih + b_hh).reshape(9, P)
    bias12[:, 0:6] = comb[0:6].T
    bias12[:, 6:9] = b_ih.reshape(9, P)[6:9].T
    bias12[:, 9:12] = b_hh.reshape(9, P)[6:9].T
    out = bias12.T.reshape(4, 3, P).transpose(1, 0, 2)
    return np.ascontiguousarray(out).astype(BF16)


def _prep_onehot(n):
    out = np.zeros((3, 3, n), np.float32)
    for k in range(3):
        out[k, k, :] = 1.0
    return out.astype(BF16)


def _prep_idxs(tokens_core):
    # child-major half-gathers: gather (g, h) covers child 7-g, parents
    # [256h, 256h+256), wrapped [16, 16] (idx i -> [i%16, i//16]) and
    # replicated over the 8 Q7 stripes.
    byc = tokens_core.reshape(P5, ARITY)  # [parent, child]
    out = np.zeros((P, 2 * ARITY, NCH // 16), np.int16)
    for g in range(ARITY):
        c = ARITY - 1 - g
        for h in range(2):
            seg = byc[h * NCH : (h + 1) * NCH, c]
            wrapped = seg.reshape(NCH // 16, 16).T.astype(np.int16)
            out[:, 2 * g + h, :] = np.tile(wrapped, (8, 1))
    return out


def _gru_step_batch(x_t, h, w_ih, w_hh, b_ih, b_hh):
    gi = x_t @ w_ih.T + b_ih
    gh = h @ w_hh.T + b_hh
    i_r, i_z, i_n = np.split(gi, 3, axis=-1)
    h_r, h_z, h_n = np.split(gh, 3, axis=-1)
    r = 1.0 / (1.0 + np.exp(-(i_r + h_r)))
    z = 1.0 / (1.0 + np.exp(-(i_z + h_z)))
    n = np.tanh(i_n + r * h_n)
    return (1.0 - z) * n + z * h


def _host_level(xs, h0, w_ih, w_hh, b_ih, b_hh):
    # xs: [n_parents, ARITY, D] child outputs in natural child order.
    h = h0
    acc = np.zeros_like(h)
    for t in range(ARITY):
        h = _gru_step_batch(xs[:, ARITY - 1 - t], h, w_ih, w_hh, b_ih, b_hh)
        acc += h
    return acc / ARITY, h


def kernel(leaf_tokens, embed_table, w_ih, w_hh, b_ih, b_hh):
    from concourse.bass_utils import run_bass_kernel_spmd

    leaf_tokens = np.asarray(leaf_tokens, np.int32)
    embed_table = np.asarray(embed_table, np.float32)
    w_ih = np.asarray(w_ih, np.float32)
    w_hh = np.asarray(w_hh, np.float32)
    b_ih = np.asarray(b_ih, np.float32)
    b_hh = np.asarray(b_hh, np.float32)

    nc = _build_program()

    # G = embed @ w_ih.T with r/z biases (b_ih+b_hh) and n bias (b_ih) folded.
    bias_fold = np.concatenate([(b_ih + b_hh)[: 2 * DIM], b_ih[2 * DIM :]])
    G = embed_table @ w_ih.T
    G += bias_fold
    G = G.astype(BF16)

    whh_t = _retile_weights(w_hh)
    wih_s = _retile_weights(w_ih / ARITY)
    biases = _prep_bias(b_ih, b_hh)
    identity = np.zeros((P, P + 3 * NCH), np.float32)
    identity[:, 0:P] = np.eye(P, dtype=np.float32)
    bhn = b_hh[2 * DIM :].reshape(3, P)  # [m, p]
    identity[:, P:] = np.repeat(bhn.T[:, :, None], NCH, axis=2).reshape(P, 3 * NCH)
    identity = identity.astype(BF16)
    threes = np.zeros((3, 3008), BF16)
    threes[:, 0:512] = _prep_biasmm(b_ih, b_hh).reshape(3, 512)
    threes[:, 512:1280] = _prep_onehot(NCH).reshape(3, 768)
    threes[:, 1280:2816] = _prep_onehot(512).reshape(3, 1536)
    threes[:, 2816:3008] = _prep_onehot(P4).reshape(3, 192)

    in_maps = []
    for core in range(NCORES):
        toks = leaf_tokens[core * LEAVES_CORE : (core + 1) * LEAVES_CORE]
        in_maps.append(
            {
                "gtab": G,
                "idxs": _prep_idxs(toks),
                "ident": identity,
                "whh_t": whh_t,
                "wih_s": wih_s,
                "biases": biases,
                "threes": threes,
            }
        )
    res = run_bass_kernel_spmd(nc, in_maps, core_ids=list(range(NCORES)))

    # ---- host epilogue: levels 3, 2 (per core) and the root ----
    w_ih64 = w_ih.astype(np.float64)
    w_hh64 = w_hh.astype(np.float64)
    b_ih64 = b_ih.astype(np.float64)
    b_hh64 = b_hh.astype(np.float64)

    x3 = np.zeros((NCORES, P4, DIM), np.float64)
    h4 = np.zeros((NCORES, P4, DIM), np.float64)
    for core in range(NCORES):
        hacc = np.asarray(res.results[core]["out_x"], np.float64)  # [128,3,64]
        hh = np.asarray(
            res.results[core]["out_h"].astype(np.float32), np.float64
        )  # [128,3,64]
        # [p, k, node] -> node-major [node, feat=128k+p]
        x3[core] = hacc.transpose(2, 1, 0).reshape(P4, DIM) / ARITY
        h4[core] = hh.transpose(2, 1, 0).reshape(P4, DIM)

    # level 3: 8 parents per core
    xs3 = x3.reshape(NCORES * ARITY, ARITY, DIM)
    h03 = h4.reshape(NCORES * ARITY, ARITY, DIM).mean(axis=1)
    x2, h3 = _host_level(xs3, h03, w_ih64, w_hh64, b_ih64, b_hh64)
    # level 2: 1 parent per core
    xs2 = x2.reshape(NCORES, ARITY, DIM)
    h02 = h3.reshape(NCORES, ARITY, DIM).mean(axis=1)
    x1, h2 = _host_level(xs2, h02, w_ih64, w_hh64, b_ih64, b_hh64)
    # root: 1 node over the 8 cores' outputs
    xs1 = x1.reshape(1, ARITY, DIM)
    h01 = h2.reshape(1, ARITY, DIM).mean(axis=1)
    x0, _ = _host_level(xs1, h01, w_ih64, w_hh64, b_ih64, b_hh64)

    return x0.astype(np.float32).reshape(1, 1, DIM)



# revision 43
# speedup vs baseline: 1.1930x; 1.0036x over previous
"""Tree-GRU (arity-8, depth-5) over embedded leaves on 8 TRN2 NeuronCores.

Sharding: data-parallel over subtrees. Each core takes 4096 contiguous leaves
and runs the two large tree levels locally (512 -> 64 parents). The three tiny
tail levels (8 -> 1 parents per core, plus the root across cores) are
latency-bound chains of 8-wide GRU steps over <=73 nodes total; they run on
host from the shipped level-4 outputs (the sharding hint's "all-gather the
last log2(M) levels" relaxation).

The leaf level never materializes embeddings on device: the input-gate
projection G = embed_table @ w_ih.T (+ folded biases) is precomputed on host
(token-independent weight prep, like the weight retiling) and the kernel
gathers G rows per leaf with dma_gather(transpose=True), which lands the rows
directly in feature-major layout [128, 9, leaves] -- no PE transposes and no
on-device input-gate matmuls for the leaf level. Tokens are pre-permuted
child-major so each gather covers exactly one (step, chunk)'s children;
gathers fan out over 4 SWDGE queues and the gpsimd queue carries nothing else
until the last gather issues.

Per step the r/z pre-activations accumulate in one 3-bank PSUM tile: gathered
gi tiles are injected with identity matmuls (region-ordered start=True bank
clears), recurrent h @ w_hh.T tiles accumulate on top, and a single sigmoid
reads the whole tile. The n-role pre-activation gets b_hh_n from a one-hot
bias matmul (the hn bank's start=True writer), so the gate chain is one
bf16 multiply and one bf16 add before the tanh. Biases ride in G (b_ih +
b_hh for r/z, b_ih for n). Hidden states live in a 3-deep ring of per-step
tiles and both hidden accumulators are parent-major, so the (slow,
late-starting) gpsimd accumulator adds run contiguous and off the chain.
"""

import numpy as np
import ml_dtypes

ARITY = 8
DIM = 384
VOCAB = 32000
NCORES = 8
P = 128
J = 3  # DIM // 128 feature tiles
N_LEAVES = 32768
LEAVES_CORE = N_LEAVES // NCORES  # 4096
P5 = LEAVES_CORE // ARITY  # 512 level-5 parents per core
P4 = P5 // ARITY  # 64 level-4 parents per core
NCH = 256  # level-5 chunk size (2 chunks)

BF16 = ml_dtypes.bfloat16

_PROG_CACHE = {}


def _emit(tc, nc, aps):
    import concourse.mybir as mybir

    f32 = mybir.dt.float32
    bf16 = mybir.dt.bfloat16
    i16 = mybir.dt.int16
    Sig = mybir.ActivationFunctionType.Sigmoid
    Tanh = mybir.ActivationFunctionType.Tanh
    Add = mybir.AluOpType.add
    Sub = mybir.AluOpType.subtract
    Mult = mybir.AluOpType.mult

    (gtab, idxs, ident_in, whh_t, wih_s, biases, threes, out_x, out_h) = aps

    from contextlib import ExitStack

    with ExitStack() as ctx:
        const = ctx.enter_context(tc.tile_pool(name="const", bufs=1))
        gpool = ctx.enter_context(tc.tile_pool(name="gpool", bufs=1))
        state = ctx.enter_context(tc.tile_pool(name="state", bufs=1))
        gates = ctx.enter_context(tc.tile_pool(name="gates", bufs=3))
        pspool = ctx.enter_context(tc.tile_pool(name="pspool", bufs=1, space="PSUM"))

        # ---- prologue: idxs first, then the 16 transposing half-gathers ----
        # ONE idx DMA: with a single completion semaphore the first gather's
        # wait stays inline on the gather op, so the auto-inserted gather-lib
        # load (~13us ucode fetch) runs ahead of the wait instead of behind
        # an idx-wait hoisted in front of it.
        idx_sb = const.tile([P, 2 * ARITY, NCH // 16], i16)
        nc.scalar.dma_start(idx_sb[:], idxs[:])

        gi_tiles = []  # [t][ch] -> [128, 9, 256]
        for g in range(ARITY):
            halves = []
            for h in range(2):
                gi = gpool.tile([P, 9, NCH], bf16, name=f"gi{g}_{h}", tag=f"gi{g}_{h}")
                nc.gpsimd.dma_gather(
                    out_ap=gi[:],
                    in_ap=gtab[:],
                    idxs_ap=idx_sb[:, 2 * g + h, :],
                    num_idxs=NCH,
                    num_idxs_reg=NCH,
                    elem_size=9 * P,
                    transpose=True,
                    queue_num=(2 * g + h) % 4,
                )
                halves.append(gi)
            gi_tiles.append(halves)

        # ---- constants / weights (overlap with gathers) ----
        # 7 DMAs total (2 idx + 5 here) so no completion-semaphore reuse.
        # The [3, x] matmul constants ride in one packed blob.
        whh_sb = const.tile([P, J, 9, P], bf16)
        wih_sb = const.tile([P, J, 9, P], bf16)
        bias_sb = const.tile([P, 12], f32)
        threes_sb = const.tile([3, 3008], bf16)
        biasmm_sb = threes_sb[:, 0:512].rearrange("p (a n) -> p a n", a=4)
        onehot_sb = threes_sb[:, 512:1280].rearrange("p (a n) -> p a n", a=3)
        onehot512_sb = threes_sb[:, 1280:2816].rearrange("p (a n) -> p a n", a=3)
        onehot4_sb = threes_sb[:, 2816:3008].rearrange("p (a n) -> p a n", a=3)
        # ident blob: [:, 0:128] identity, [:, 128:896] b_hh_n broadcast over
        # NCH cols per m-tile (rhs of the hn bias id-MMs -- reuses the
        # already-loaded identity stationary, no biasmm LDWEIGHTS)
        identb = const.tile([P, P + 3 * NCH], bf16)
        ident = identb[:, 0:P]
        bvec = identb[:, P:].rearrange("p (a n) -> p a n", a=3)
        nc.sync.dma_start(bias_sb[:], biases[:])
        nc.sync.dma_start(identb[:], ident_in[:])
        nc.scalar.dma_start(threes_sb[:], threes[:])
        nc.sync.dma_start(whh_sb[:], whh_t[:])
        nc.sync.dma_start(wih_sb[:], wih_s[:])

        # ---- state ----
        # bf16: the accumulate then runs in the DVE's 2x mode (~560ns vs
        # ~950ns f32); the ~0.4%-per-add rounding is well inside tolerance
        hacc5 = state.tile([P, J, P5], bf16, name="hacc5", tag="hacc5")
        csum5 = state.tile([P, J, P4], f32, name="csum5", tag="csum5")
        # chunk-major: x4[:, j, ch] is written contiguously when level-5
        # chunk ch of step 7 retires, letting the gi4 matmuls for ch=0
        # start while ch=1 is still computing
        x4 = state.tile([P, J, 2, ARITY, P4 // 2], bf16, name="x4", tag="x4")
        # per-role gi4 tiles: level-4 step 0 needs only r/z for its id-MMs,
        # so it can start before the n-role copies land
        gi4_roles = [
            state.tile([P, 3, ARITY, P4], bf16, name=f"gi4_{r}", tag=f"gi4_{r}")
            for r in range(3)
        ]
        hacc4 = state.tile([P, J, P4], f32, name="hacc4", tag="hacc4")
        nc.vector.memset(hacc5[:], 0.0)
        nc.vector.memset(hacc4[:], 0.0)

        def rz_tile():
            # 3 PSUM banks; regions j=0..5 of NCH cols (bank j//2, half j%2)
            t_ = pspool.tile([P, J, 512], f32, name="rzps", tag="rzps", bufs=2)
            return t_, t_.rearrange("p a (b n) -> p (a b) n", b=2)

        def hn_tile():
            # 2 PSUM banks; regions m=0..2 of NCH cols in banks 0..1
            t_ = pspool.tile([P, 2, 512], f32, name="hnps", tag="hnps", bufs=1)
            return t_, t_.rearrange("p a (b n) -> p (a b) n", b=2)[:, 0:3]

        # ================= level 5: 512 parents, 2 chunks of 256 =============
        with nc.named_scope("level5"):
            h_prev = [None, None]
            for t in range(ARITY):
                h_list = []
                for ch in range(P5 // NCH):
                    sl = slice(ch * NCH, (ch + 1) * NCH)
                    gi = gi_tiles[t][ch]
                    gi_n = gi[:, 6:9, :]

                    # per-chunk h tile: contiguous reads everywhere and
                    # chunk-level readiness for the next step's matmuls.
                    # Deep ring (6): the gpsimd hacc reader lags ~5 steps
                    # behind while the standard-ucode lib loads mid-level,
                    # and h slots must not recycle through it.
                    h_new = gates.tile(
                        [P, J, NCH],
                        bf16,
                        name=f"h5_{ch}",
                        tag=f"h5_{ch}",
                        bufs=6,
                    )
                    def g5(name):
                        return gates.tile(
                            [P, J, NCH], bf16, name=name, tag=name
                        )

                    r_sb = g5("r5")
                    z_sb = g5("z5")
                    rhn = g5("rhn")
                    npre = g5("npre")
                    n_sb = g5("nsb")
                    t1 = g5("t1")
                    zh = g5("zh")
                    zm1 = g5("zm1")

                    if t == 0:
                        # h0 = 0: pre_rz = gi (biases folded in G); no PSUM.
                        # z_sb here holds (1-z) directly.
                        nc.scalar.activation(r_sb[:], gi[:, 0:3, :], Sig)
                        nc.scalar.activation(
                            z_sb[:], gi[:, 3:6, :], Sig, scale=-1.0
                        )
                        for m in range(J):
                            # rhn = r * b_hh_n + gi_n
                            nc.vector.scalar_tensor_tensor(
                                out=npre[:, m],
                                in0=r_sb[:, m],
                                scalar=bias_sb[:, 6 + m : 7 + m],
                                in1=gi_n[:, m],
                                op0=Mult,
                                op1=Add,
                            )
                        nc.scalar.activation(n_sb[:], npre[:], Tanh)
                        # h = (1-z)*n = w*n  (h0 = 0)
                        nc.vector.tensor_tensor(
                            out=h_new[:], in0=z_sb[:], in1=n_sb[:], op=Mult
                        )
                    else:
                        hp = h_prev[ch]
                        # rz: one 3-bank tile; per bank one full-bank id-MM
                        # (N=512, start=True) injects gi, then hh accumulates.
                        # All six rz regions fill BEFORE hn: readiness is
                        # tile-level, so sig(r) waits every rzps writer --
                        # putting hn last keeps it off sig(r)'s gate while
                        # it overlaps with sig(r) itself.
                        rzp, rzv = rz_tile()
                        for b in range(3):
                            nc.tensor.matmul(
                                rzp[:, b, :],
                                ident[:],
                                gi[:, 2 * b : 2 * b + 2, :].rearrange(
                                    "p a n -> p (a n)"
                                ),
                                start=True,
                                stop=False,
                            )
                        for j in (0, 1, 2, 3):
                            for k in range(J):
                                nc.tensor.matmul(
                                    rzv[:, j],
                                    whh_sb[:, k, j, :],
                                    hp[:, k, :],
                                    start=False,
                                    stop=(j % 2 == 1 and k == 2),
                                )
                        for j in (4, 5):
                            for k in range(J):
                                nc.tensor.matmul(
                                    rzv[:, j],
                                    whh_sb[:, k, j, :],
                                    hp[:, k, :],
                                    start=False,
                                    stop=(j == 5 and k == 2),
                                )
                        # hn: 2 banks; b_hh_n enters via identity matmuls on
                        # a broadcast-constant rhs (same ident stationary as
                        # the rz injections -- no extra LDWEIGHTS), each
                        # bank's start=True writer; hh accumulates. Keeping
                        # rhn a SINGLE vector op matters: the hn psum ring
                        # is bufs=1, so the bank is held until rhn's last
                        # read -- a spread 3-op STT here stalls the next
                        # chunk's hn matmuls on the in-order PE queue.
                        hnp, hn_v = hn_tile()
                        nc.tensor.matmul(
                            hnp[:, 0, :],
                            ident[:],
                            bvec[:, 0:2, :].rearrange("p a n -> p (a n)"),
                            start=True,
                            stop=False,
                        )
                        nc.tensor.matmul(
                            hnp[:, 1, :NCH],
                            ident[:],
                            bvec[:, 2, :],
                            start=True,
                            stop=False,
                        )
                        for m in range(J):
                            for k in range(J):
                                nc.tensor.matmul(
                                    hn_v[:, m],
                                    whh_sb[:, k, 6 + m, :],
                                    hp[:, k, :],
                                    start=False,
                                    stop=(k == 2 and m != 0),
                                )

                        nc.scalar.activation(r_sb[:], rzv[:, 0:3], Sig)
                        nc.scalar.activation(z_sb[:], rzv[:, 3:6], Sig)
                        # chain: rhn -> npre -> tanh -> t1 -> h; zh rides the
                        # DVE queue between npre and t1 (its data is ready
                        # early, and it fills the tanh window).
                        nc.vector.tensor_tensor(
                            out=rhn[:], in0=hn_v, in1=r_sb[:], op=Mult
                        )
                        nc.vector.tensor_tensor(
                            out=npre[:], in0=rhn[:], in1=gi_n, op=Add
                        )
                        nc.vector.tensor_tensor(
                            out=zh[:], in0=z_sb[:], in1=hp[:], op=Mult
                        )
                        nc.scalar.activation(zm1[:], rzv[:, 3:6], Sig, scale=-1.0)
                        nc.scalar.activation(n_sb[:], npre[:], Tanh)
                        nc.vector.tensor_tensor(
                            out=t1[:], in0=zm1[:], in1=n_sb[:], op=Mult
                        )
                        nc.vector.tensor_tensor(
                            out=h_new[:], in0=zh[:], in1=t1[:], op=Add
                        )

                    h_list.append(h_new)
                    if t == ARITY - 1:
                        qsl = slice(ch * NCH // ARITY, (ch + 1) * NCH // ARITY)
                        # child-mean of final hiddens -> h0 of level 4
                        nc.vector.tensor_reduce(
                            out=csum5[:, :, qsl],
                            in_=h_new.rearrange("p j (q c) -> p j q c", c=ARITY),
                            axis=mybir.AxisListType.X,
                            op=Add,
                        )
                        # x4 = hacc + h (raw sum; /8 folded into wih_s)
                        hperm = h_new.rearrange("p j (q c) -> p j c q", c=ARITY)
                        for j in range(J):
                            nc.vector.tensor_tensor(
                                out=x4[:, j, ch],
                                in0=hacc5[:, j, sl].rearrange(
                                    "p (q c) -> p c q", c=ARITY
                                ),
                                in1=hperm[:, j],
                                op=Add,
                            )
                    else:
                        # parent-major contiguous accumulate. On the DVE,
                        # NOT gpsimd: gpsimd TTs here both queue behind the
                        # gather descgens (lib batching) and visibly slow
                        # concurrent DVE ops 2-3x while running.
                        nc.vector.tensor_tensor(
                            out=hacc5[:, :, sl],
                            in0=hacc5[:, :, sl],
                            in1=h_new[:],
                            op=Add,
                        )
                h_prev = h_list

        # ================= level 4: 64 parents, single chunk =================
        with nc.named_scope("level4"):
            h4 = gates.tile([P, J, P4], bf16, name="h4", tag="h4")
            nc.scalar.mul(h4[:], csum5[:], 1.0 / ARITY)

            # gi4 = x4 @ (w_ih/8).T + biases: one 3-bank group per role
            # (regions j' = bank, N=512 over all (child, parent) columns).
            # psum->sbuf copies split across scalar/vector so they drain in
            # ~2 copy-times, not 3 serial on the vector engine.
            # biases enter as per-partition adds on the psum->sbuf copies
            # (gi4 is feature-major, so the bias is constant per partition
            # within each m-tile) -- no bias matmuls on the PE. Roles r/z
            # run first (level-4 step 0 needs them for its id-MMs), each in
            # a ch0 phase (overlaps step 7 chunk 1) then a ch1 phase; the
            # n-role rides the ring afterwards.
            bias_col = {0: 0, 1: 3, 2: 9}

            def gi4_mms(role, rp, ch):
                for jj in range(3):
                    for k in range(J):
                        nc.tensor.matmul(
                            rp[:, jj, NCH * ch : NCH * (ch + 1)],
                            wih_sb[:, k, 3 * role + jj, :],
                            x4[:, k, ch].rearrange("p c q -> p (c q)"),
                            start=(ch == 0 and k == 0),
                            stop=(k == 2),
                        )

            def gi4_copies(role, rp):
                for m in range(3):
                    src = rp[:, m, :].rearrange(
                        "p (ch c q) -> p ch c q", ch=2, c=ARITY
                    )
                    dst = gi4_roles[role][:, m].rearrange(
                        "p c (ch q) -> p ch c q", ch=2
                    )
                    bcol = bias_col[role] + m
                    if (role + m) % 2 == 0:
                        nc.scalar.add(
                            out=dst, in_=src, add=bias_sb[:, bcol : bcol + 1]
                        )
                    else:
                        nc.vector.tensor_scalar_add(
                            out=dst,
                            in0=src,
                            scalar1=bias_sb[:, bcol : bcol + 1],
                        )

            rzpA, _ = rz_tile()
            rzpB, _ = rz_tile()
            for ch in (0, 1):
                gi4_mms(0, rzpA, ch)
                gi4_mms(1, rzpB, ch)
            gi4_copies(0, rzpA)
            gi4_copies(1, rzpB)
            rzpC, _ = rz_tile()
            for ch in (0, 1):
                gi4_mms(2, rzpC, ch)
            gi4_copies(2, rzpC)

            for t in range(ARITY):
                c = ARITY - 1 - t
                gi_n = gi4_roles[2][:, :, c, :]

                r_sb = gates.tile([P, J, P4], bf16, name="r4", tag="r4")
                z_sb = gates.tile([P, J, P4], bf16, name="z4", tag="z4")
                rhn = gates.tile([P, J, P4], bf16, name="rhn4", tag="rhn4")
                npre = gates.tile([P, J, P4], bf16, name="npre4", tag="npre4")
                n_sb = gates.tile([P, J, P4], bf16, name="nsb4", tag="nsb4")
                t1 = gates.tile([P, J, P4], bf16, name="t14", tag="t14")
                zh4 = gates.tile([P, J, P4], bf16, name="zh4", tag="zh4")
                zm14 = gates.tile([P, J, P4], bf16, name="zm14", tag="zm14")
                h4n = gates.tile([P, J, P4], bf16, name="h4", tag="h4")

                # r, hn, and z each fill their OWN psum tile instance so
                # readers wait only their own writers: sig(r) fires after the
                # 10 r-side MMs, not the whole 30-MM burst. id-MMs first
                # (h-independent), then hh r, hn, hh z.
                rzpA, _ = rz_tile()
                rv = rzpA[:, 0, :].rearrange("p (m n) -> p m n", m=8)[:, 0:3]
                nc.tensor.matmul(
                    rzpA[:, 0, : 3 * P4].rearrange("p (a n) -> p a n", a=3),
                    ident[:],
                    gi4_roles[0][:, :, c, :],
                    start=True,
                    stop=False,
                )
                for m in range(J):
                    for k in range(J):
                        nc.tensor.matmul(
                            rv[:, m],
                            whh_sb[:, k, m, :],
                            h4[:, k, :],
                            start=False,
                            stop=(m == 2 and k == 2),
                        )
                hnp, _ = hn_tile()
                hn_v = hnp[:, 0, :].rearrange("p (m n) -> p m n", m=8)[:, 0:3]
                nc.tensor.matmul(
                    hnp[:, 0, : 3 * P4],
                    biasmm_sb[:, 3, :],
                    onehot4_sb[:, :, :].rearrange("k m n -> k (m n)"),
                    start=True,
                    stop=False,
                )
                for m in range(J):
                    for k in range(J):
                        nc.tensor.matmul(
                            hn_v[:, m],
                            whh_sb[:, k, 6 + m, :],
                            h4[:, k, :],
                            start=False,
                            stop=(m == 2 and k == 2),
                        )
                rzpB, _ = rz_tile()
                zv = rzpB[:, 0, :].rearrange("p (m n) -> p m n", m=8)[:, 0:3]
                nc.tensor.matmul(
                    rzpB[:, 0, : 3 * P4].rearrange("p (a n) -> p a n", a=3),
                    ident[:],
                    gi4_roles[1][:, :, c, :],
                    start=True,
                    stop=False,
                )
                for m in range(J):
                    for k in range(J):
                        nc.tensor.matmul(
                            zv[:, m],
                            whh_sb[:, k, 3 + m, :],
                            h4[:, k, :],
                            start=False,
                            stop=(m == 2 and k == 2),
                        )

                nc.scalar.activation(r_sb[:], rv, Sig)
                nc.scalar.activation(z_sb[:], zv, Sig)
                nc.vector.tensor_tensor(out=rhn[:], in0=hn_v, in1=r_sb[:], op=Mult)
                nc.vector.tensor_tensor(out=npre[:], in0=rhn[:], in1=gi_n, op=Add)
                nc.vector.tensor_tensor(out=zh4[:], in0=z_sb[:], in1=h4[:], op=Mult)
                nc.scalar.activation(zm14[:], zv, Sig, scale=-1.0)
                nc.scalar.activation(n_sb[:], npre[:], Tanh)
                nc.vector.tensor_tensor(out=t1[:], in0=zm14[:], in1=n_sb[:], op=Mult)
                nc.vector.tensor_tensor(out=h4n[:], in0=zh4[:], in1=t1[:], op=Add)
                # hacc runs off-chain on the (idle) gpsimd engine
                nc.gpsimd.tensor_tensor(
                    out=hacc4[:], in0=hacc4[:], in1=h4n[:], op=Add
                )
                h4 = h4n

            # ---- ship level-4 outputs ----
            nc.sync.dma_start(out_x[:], hacc4[:])
            nc.sync.dma_start(out_h[:], h4[:])


def _build_program():
    if "nc" in _PROG_CACHE:
        return _PROG_CACHE["nc"]
    import concourse.bacc as bacc
    import concourse.mybir as mybir
    import concourse.tile as tile

    f32 = mybir.dt.float32
    bf16 = mybir.dt.bfloat16

    nc = bacc.Bacc(
        "TRN2",
        target_bir_lowering=False,
        debug=False,
        enable_asserts=False,
        num_devices=NCORES,
        num_swdge_queues=4,
    )
    gtab = nc.dram_tensor("gtab", [VOCAB, 9 * P], bf16, kind="ExternalInput").ap()
    idxs = nc.dram_tensor(
        "idxs", [P, 2 * ARITY, NCH // 16], mybir.dt.int16, kind="ExternalInput"
    ).ap()
    ident_in = nc.dram_tensor(
        "ident", [P, P + 3 * NCH], bf16, kind="ExternalInput"
    ).ap()
    whh_t = nc.dram_tensor("whh_t", [P, J, 9, P], bf16, kind="ExternalInput").ap()
    wih_s = nc.dram_tensor("wih_s", [P, J, 9, P], bf16, kind="ExternalInput").ap()
    biases = nc.dram_tensor("biases", [P, 12], f32, kind="ExternalInput").ap()
    threes = nc.dram_tensor("threes", [3, 3008], bf16, kind="ExternalInput").ap()
    out_x = nc.dram_tensor("out_x", [P, J, P4], f32, kind="ExternalOutput").ap()
    out_h = nc.dram_tensor("out_h", [P, J, P4], bf16, kind="ExternalOutput").ap()

    with tile.TileContext(nc) as tc:
        _emit(
            tc,
            nc,
            (gtab, idxs, ident_in, whh_t, wih_s, biases, threes, out_x, out_h),
        )
    nc.compile()
    _PROG_CACHE["nc"] = nc
    return nc


def _retile_weights(w):
    # w: [1152, 384] -> lhsT tiles [128(k_part), 3(k), 9(m), 128(m_col)] bf16
    wt = np.ascontiguousarray(w.T)  # [384, 1152]
    wt = wt.reshape(J, P, 9, P).transpose(1, 0, 2, 3)
    return np.ascontiguousarray(wt).astype(BF16)


def _prep_bias(b_ih, b_hh):
    biases = np.zeros((P, 12), np.float32)
    comb = (b_ih + b_hh).reshape(9, P)
    biases[:, 0:6] = comb[0:6].T
    biases[:, 6:9] = b_hh.reshape(9, P)[6:9].T
    biases[:, 9:12] = b_ih.reshape(9, P)[6:9].T
    return biases


def _prep_biasmm(b_ih, b_hh):
    # biasmm[k, role, p]: roles 0..2 feed the gi4 bias matmuls (combined r/z
    # bias and b_ih_n); role 3 is b_hh_n for the hn-bank bias matmuls.
    bias12 = np.zeros((P, 12), np.float32)
    comb = (b_ih + b_hh).reshape(9, P)
    bias12[:, 0:6] = comb[0:6].T
    bias12[:, 6:9] = b_ih.reshape(9, P)[6:9].T
    bias12[:, 9:12] = b_hh.reshape(9, P)[6:9].T
    out = bias12.T.reshape(4, 3, P).transpose(1, 0, 2)
    return np.ascontiguousarray(out).astype(BF16)


def _prep_onehot(n):
    out = np.zeros((3, 3, n), np.float32)
    for k in range(3):
        out[k, k, :] = 1.0
    return out.astype(BF16)


def _prep_idxs(tokens_core):
    # child-major half-gathers: gather (g, h) covers child 7-g, parents
    # [256h, 256h+256), wrapped [16, 16] (idx i -> [i%16, i//16]) and
    # replicated over the 8 Q7 stripes.
    byc = tokens_core.reshape(P5, ARITY)  # [parent, child]
    out = np.zeros((P, 2 * ARITY, NCH // 16), np.int16)
    for g in range(ARITY):
        c = ARITY - 1 - g
        for h in range(2):
            seg = byc[h * NCH : (h + 1) * NCH, c]
            wrapped = seg.reshape(NCH // 16, 16).T.astype(np.int16)
            out[:, 2 * g + h, :] = np.tile(wrapped, (8, 1))
    return out


def _gru_step_batch(x_t, h, w_ih, w_hh, b_ih, b_hh):
    gi = x_t @ w_ih.T + b_ih
    gh = h @ w_hh.T + b_hh
    i_r, i_z, i_n = np.split(gi, 3, axis=-1)
    h_r, h_z, h_n = np.split(gh, 3, axis=-1)
    r = 1.0 / (1.0 + np.exp(-(i_r + h_r)))
    z = 1.0 / (1.0 + np.exp(-(i_z + h_z)))
    n = np.tanh(i_n + r * h_n)
    return (1.0 - z) * n + z * h


def _host_level(xs, h0, w_ih, w_hh, b_ih, b_hh):
    # xs: [n_parents, ARITY, D] child outputs in natural child order.
    h = h0
    acc = np.zeros_like(h)
    for t in range(ARITY):
        h = _gru_step_batch(xs[:, ARITY - 1 - t], h, w_ih, w_hh, b_ih, b_hh)
        acc += h
    return acc / ARITY, h


def kernel(leaf_tokens, embed_table, w_ih, w_hh, b_ih, b_hh):
    from concourse.bass_utils import run_bass_kernel_spmd

    leaf_tokens = np.asarray(leaf_tokens, np.int32)
    embed_table = np.asarray(embed_table, np.float32)
    w_ih = np.asarray(w_ih, np.float32)
    w_hh = np.asarray(w_hh, np.float32)
    b_ih = np.asarray(b_ih, np.float32)
    b_hh = np.asarray(b_hh, np.float32)

    nc = _build_program()

    # G = embed @ w_ih.T with r/z biases (b_ih+b_hh) and n bias (b_ih) folded.
    bias_fold = np.concatenate([(b_ih + b_hh)[: 2 * DIM], b_ih[2 * DIM :]])
    G = embed_table @ w_ih.T
    G += bias_fold
    G = G.astype(BF16)

    whh_t = _retile_weights(w_hh)
    wih_s = _retile_weights(w_ih / ARITY)
    biases = _prep_bias(b_ih, b_hh)
    identity = np.zeros((P, P + 3 * NCH), np.float32)
    identity[:, 0:P] = np.eye(P, dtype=np.float32)
    bhn = b_hh[2 * DIM :].reshape(3, P)  # [m, p]
    identity[:, P:] = np.repeat(bhn.T[:, :, None], NCH, axis=2).reshape(P, 3 * NCH)
    identity = identity.astype(BF16)
    threes = np.zeros((3, 3008), BF16)
    threes[:, 0:512] = _prep_biasmm(b_ih, b_hh).reshape(3, 512)
    threes[:, 512:1280] = _prep_onehot(NCH).reshape(3, 768)
    threes[:, 1280:2816] = _prep_onehot(512).reshape(3, 1536)
    threes[:, 2816:3008] = _prep_onehot(P4).reshape(3, 192)

    in_maps = []
    for core in range(NCORES):
        toks = leaf_tokens[core * LEAVES_CORE : (core + 1) * LEAVES_CORE]
        in_maps.append(
            {
                "gtab": G,
                "idxs": _prep_idxs(toks),
                "ident": identity,
                "whh_t": whh_t,
                "wih_s": wih_s,
                "biases": biases,
                "threes": threes,
            }
        )
    res = run_bass_kernel_spmd(nc, in_maps, core_ids=list(range(NCORES)))

    # ---- host epilogue: levels 3, 2 (per core) and the root ----
    w_ih64 = w_ih.astype(np.float64)
    w_hh64 = w_hh.astype(np.float64)
    b_ih64 = b_ih.astype(np.float64)
    b_hh64 = b_hh.astype(np.float64)

    x3 = np.zeros((NCORES, P4, DIM), np.float64)
    h4 = np.zeros((NCORES, P4, DIM), np.float64)
    for core in range(NCORES):
        hacc = np.asarray(res.results[core]["out_x"], np.float64)  # [128,3,64]
        hh = np.asarray(
            res.results[core]["out_h"].astype(np.float32), np.float64
        )  # [128,3,64]
        # [p, k, node] -> node-major [node, feat=128k+p]
        x3[core] = hacc.transpose(2, 1, 0).reshape(P4, DIM) / ARITY
        h4[core] = hh.transpose(2, 1, 0).reshape(P4, DIM)

    # level 3: 8 parents per core
    xs3 = x3.reshape(NCORES * ARITY, ARITY, DIM)
    h03 = h4.reshape(NCORES * ARITY, ARITY, DIM).mean(axis=1)
    x2, h3 = _host_level(xs3, h03, w_ih64, w_hh64, b_ih64, b_hh64)
    # level 2: 1 parent per core
    xs2 = x2.reshape(NCORES, ARITY, DIM)
    h02 = h3.reshape(NCORES, ARITY, DIM).mean(axis=1)
    x1, h2 = _host_level(xs2, h02, w_ih64, w_hh64, b_ih64, b_hh64)
    # root: 1 node over the 8 cores' outputs
    xs1 = x1.reshape(1, ARITY, DIM)
    h01 = h2.reshape(1, ARITY, DIM).mean(axis=1)
    x0, _ = _host_level(xs1, h01, w_ih64, w_hh64, b_ih64, b_hh64)

    return x0.astype(np.float32).reshape(1, 1, DIM)



# revision 44
# speedup vs baseline: 1.2133x; 1.0170x over previous
"""Tree-GRU (arity-8, depth-5) over embedded leaves on 8 TRN2 NeuronCores.

Sharding: data-parallel over subtrees. Each core takes 4096 contiguous leaves
and runs the two large tree levels locally (512 -> 64 parents). The three tiny
tail levels (8 -> 1 parents per core, plus the root across cores) are
latency-bound chains of 8-wide GRU steps over <=73 nodes total; they run on
host from the shipped level-4 outputs (the sharding hint's "all-gather the
last log2(M) levels" relaxation).

The leaf level never materializes embeddings on device: the input-gate
projection G = embed_table @ w_ih.T (+ folded biases) is precomputed on host
(token-independent weight prep, like the weight retiling) and the kernel
gathers G rows per leaf with dma_gather(transpose=True), which lands the rows
directly in feature-major layout [128, 9, leaves] -- no PE transposes and no
on-device input-gate matmuls for the leaf level. Tokens are pre-permuted
child-major so each gather covers exactly one (step, chunk)'s children;
gathers fan out over 4 SWDGE queues and the gpsimd queue carries nothing else
until the last gather issues.

Per step the r/z pre-activations accumulate in one 3-bank PSUM tile: gathered
gi tiles are injected with identity matmuls (region-ordered start=True bank
clears), recurrent h @ w_hh.T tiles accumulate on top, and a single sigmoid
reads the whole tile. The n-role pre-activation gets b_hh_n from a one-hot
bias matmul (the hn bank's start=True writer), so the gate chain is one
bf16 multiply and one bf16 add before the tanh. Biases ride in G (b_ih +
b_hh for r/z, b_ih for n). Hidden states live in a 3-deep ring of per-step
tiles and both hidden accumulators are parent-major, so the (slow,
late-starting) gpsimd accumulator adds run contiguous and off the chain.
"""

import numpy as np
import ml_dtypes

ARITY = 8
DIM = 384
VOCAB = 32000
NCORES = 8
P = 128
J = 3  # DIM // 128 feature tiles
N_LEAVES = 32768
LEAVES_CORE = N_LEAVES // NCORES  # 4096
P5 = LEAVES_CORE // ARITY  # 512 level-5 parents per core
P4 = P5 // ARITY  # 64 level-4 parents per core
NCH = 256  # level-5 chunk size (2 chunks)

BF16 = ml_dtypes.bfloat16

_PROG_CACHE = {}


def _emit(tc, nc, aps):
    import concourse.mybir as mybir

    f32 = mybir.dt.float32
    bf16 = mybir.dt.bfloat16
    i16 = mybir.dt.int16
    Sig = mybir.ActivationFunctionType.Sigmoid
    Tanh = mybir.ActivationFunctionType.Tanh
    Add = mybir.AluOpType.add
    Sub = mybir.AluOpType.subtract
    Mult = mybir.AluOpType.mult

    (gtab, idxs, ident_in, whh_t, wih_s, biases, threes, out_x, out_h) = aps

    from contextlib import ExitStack

    with ExitStack() as ctx:
        const = ctx.enter_context(tc.tile_pool(name="const", bufs=1))
        gpool = ctx.enter_context(tc.tile_pool(name="gpool", bufs=1))
        state = ctx.enter_context(tc.tile_pool(name="state", bufs=1))
        gates = ctx.enter_context(tc.tile_pool(name="gates", bufs=3))
        pspool = ctx.enter_context(tc.tile_pool(name="pspool", bufs=1, space="PSUM"))

        # ---- prologue: idxs first, then the 16 transposing half-gathers ----
        # ONE idx DMA: with a single completion semaphore the first gather's
        # wait stays inline on the gather op, so the auto-inserted gather-lib
        # load (~13us ucode fetch) runs ahead of the wait instead of behind
        # an idx-wait hoisted in front of it.
        idx_sb = const.tile([P, 2 * ARITY, NCH // 16], i16)
        nc.scalar.dma_start(idx_sb[:], idxs[:])

        gi_tiles = []  # [t][ch] -> [128, 9, 256]
        for g in range(ARITY):
            halves = []
            for h in range(2):
                gi = gpool.tile([P, 9, NCH], bf16, name=f"gi{g}_{h}", tag=f"gi{g}_{h}")
                nc.gpsimd.dma_gather(
                    out_ap=gi[:],
                    in_ap=gtab[:],
                    idxs_ap=idx_sb[:, 2 * g + h, :],
                    num_idxs=NCH,
                    num_idxs_reg=NCH,
                    elem_size=9 * P,
                    transpose=True,
                    queue_num=(2 * g + h) % 4,
                )
                halves.append(gi)
            gi_tiles.append(halves)

        # ---- constants / weights (overlap with gathers) ----
        # 7 DMAs total (2 idx + 5 here) so no completion-semaphore reuse.
        # The [3, x] matmul constants ride in one packed blob.
        whh_sb = const.tile([P, J, 9, P], bf16)
        wih_sb = const.tile([P, J, 9, P], bf16)
        bias_sb = const.tile([P, 12], f32)
        threes_sb = const.tile([3, 3008], bf16)
        biasmm_sb = threes_sb[:, 0:512].rearrange("p (a n) -> p a n", a=4)
        onehot_sb = threes_sb[:, 512:1280].rearrange("p (a n) -> p a n", a=3)
        onehot512_sb = threes_sb[:, 1280:2816].rearrange("p (a n) -> p a n", a=3)
        onehot4_sb = threes_sb[:, 2816:3008].rearrange("p (a n) -> p a n", a=3)
        # ident blob: [:, 0:128] identity, [:, 128:896] b_hh_n broadcast over
        # NCH cols per m-tile (rhs of the hn bias id-MMs -- reuses the
        # already-loaded identity stationary, no biasmm LDWEIGHTS)
        identb = const.tile([P, P + 3 * NCH], bf16)
        ident = identb[:, 0:P]
        bvec = identb[:, P:].rearrange("p (a n) -> p a n", a=3)
        nc.sync.dma_start(bias_sb[:], biases[:])
        nc.sync.dma_start(identb[:], ident_in[:])
        nc.scalar.dma_start(threes_sb[:], threes[:])
        nc.sync.dma_start(whh_sb[:], whh_t[:])
        nc.sync.dma_start(wih_sb[:], wih_s[:])

        # ---- state ----
        # bf16: the accumulate then runs in the DVE's 2x mode (~560ns vs
        # ~950ns f32); the ~0.4%-per-add rounding is well inside tolerance
        hacc5 = state.tile([P, J, P5], bf16, name="hacc5", tag="hacc5")
        csum5 = state.tile([P, J, P4], f32, name="csum5", tag="csum5")
        # chunk-major: x4[:, j, ch] is written contiguously when level-5
        # chunk ch of step 7 retires, letting the gi4 matmuls for ch=0
        # start while ch=1 is still computing
        x4 = state.tile([P, J, 2, ARITY, P4 // 2], bf16, name="x4", tag="x4")
        # per-role gi4 tiles: level-4 step 0 needs only r/z for its id-MMs,
        # so it can start before the n-role copies land
        gi4_roles = [
            state.tile([P, 3, ARITY, P4], bf16, name=f"gi4_{r}", tag=f"gi4_{r}")
            for r in range(3)
        ]
        hacc4 = state.tile([P, J, P4], f32, name="hacc4", tag="hacc4")
        nc.vector.memset(hacc5[:], 0.0)
        nc.vector.memset(hacc4[:], 0.0)

        def rz_tile():
            # 3 PSUM banks; regions j=0..5 of NCH cols (bank j//2, half j%2)
            t_ = pspool.tile([P, J, 512], f32, name="rzps", tag="rzps", bufs=2)
            return t_, t_.rearrange("p a (b n) -> p (a b) n", b=2)

        def hn_tile():
            # 2 PSUM banks; regions m=0..2 of NCH cols in banks 0..1
            t_ = pspool.tile([P, 2, 512], f32, name="hnps", tag="hnps", bufs=1)
            return t_, t_.rearrange("p a (b n) -> p (a b) n", b=2)[:, 0:3]

        # ================= level 5: 512 parents, 2 chunks of 256 =============
        with nc.named_scope("level5"):
            h_prev = [None, None]
            for t in range(ARITY):
                h_list = []
                for ch in range(P5 // NCH):
                    sl = slice(ch * NCH, (ch + 1) * NCH)
                    gi = gi_tiles[t][ch]
                    gi_n = gi[:, 6:9, :]

                    # per-chunk h tile: contiguous reads everywhere and
                    # chunk-level readiness for the next step's matmuls.
                    # Deep ring (6): the gpsimd hacc reader lags ~5 steps
                    # behind while the standard-ucode lib loads mid-level,
                    # and h slots must not recycle through it.
                    h_new = gates.tile(
                        [P, J, NCH],
                        bf16,
                        name=f"h5_{ch}",
                        tag=f"h5_{ch}",
                        bufs=6,
                    )
                    def g5(name):
                        return gates.tile(
                            [P, J, NCH], bf16, name=name, tag=name
                        )

                    r_sb = g5("r5")
                    z_sb = g5("z5")
                    rhn = g5("rhn")
                    npre = g5("npre")
                    n_sb = g5("nsb")
                    t1 = g5("t1")
                    zh = g5("zh")
                    zm1 = g5("zm1")

                    if t == 0:
                        # h0 = 0: pre_rz = gi (biases folded in G); no PSUM.
                        # z_sb here holds (1-z) directly.
                        nc.scalar.activation(r_sb[:], gi[:, 0:3, :], Sig)
                        nc.scalar.activation(
                            z_sb[:], gi[:, 3:6, :], Sig, scale=-1.0
                        )
                        for m in range(J):
                            # rhn = r * b_hh_n + gi_n
                            nc.vector.scalar_tensor_tensor(
                                out=npre[:, m],
                                in0=r_sb[:, m],
                                scalar=bias_sb[:, 6 + m : 7 + m],
                                in1=gi_n[:, m],
                                op0=Mult,
                                op1=Add,
                            )
                        nc.scalar.activation(n_sb[:], npre[:], Tanh)
                        # h = (1-z)*n = w*n  (h0 = 0)
                        nc.vector.tensor_tensor(
                            out=h_new[:], in0=z_sb[:], in1=n_sb[:], op=Mult
                        )
                    else:
                        hp = h_prev[ch]
                        # rz: one 3-bank tile; per bank one full-bank id-MM
                        # (N=512, start=True) injects gi, then hh accumulates.
                        # All six rz regions fill BEFORE hn: readiness is
                        # tile-level, so sig(r) waits every rzps writer --
                        # putting hn last keeps it off sig(r)'s gate while
                        # it overlaps with sig(r) itself.
                        rzp, rzv = rz_tile()
                        for b in range(3):
                            nc.tensor.matmul(
                                rzp[:, b, :],
                                ident[:],
                                gi[:, 2 * b : 2 * b + 2, :].rearrange(
                                    "p a n -> p (a n)"
                                ),
                                start=True,
                                stop=False,
                            )
                        for j in (0, 1, 2, 3):
                            for k in range(J):
                                nc.tensor.matmul(
                                    rzv[:, j],
                                    whh_sb[:, k, j, :],
                                    hp[:, k, :],
                                    start=False,
                                    stop=(j % 2 == 1 and k == 2),
                                )
                        for j in (4, 5):
                            for k in range(J):
                                nc.tensor.matmul(
                                    rzv[:, j],
                                    whh_sb[:, k, j, :],
                                    hp[:, k, :],
                                    start=False,
                                    stop=(j == 5 and k == 2),
                                )
                        # hn: 2 banks; b_hh_n enters via identity matmuls on
                        # a broadcast-constant rhs (same ident stationary as
                        # the rz injections -- no extra LDWEIGHTS), each
                        # bank's start=True writer; hh accumulates. Keeping
                        # rhn a SINGLE vector op matters: the hn psum ring
                        # is bufs=1, so the bank is held until rhn's last
                        # read -- a spread 3-op STT here stalls the next
                        # chunk's hn matmuls on the in-order PE queue.
                        hnp, hn_v = hn_tile()
                        nc.tensor.matmul(
                            hnp[:, 0, :],
                            ident[:],
                            bvec[:, 0:2, :].rearrange("p a n -> p (a n)"),
                            start=True,
                            stop=False,
                        )
                        nc.tensor.matmul(
                            hnp[:, 1, :NCH],
                            ident[:],
                            bvec[:, 2, :],
                            start=True,
                            stop=False,
                        )
                        for m in range(J):
                            for k in range(J):
                                nc.tensor.matmul(
                                    hn_v[:, m],
                                    whh_sb[:, k, 6 + m, :],
                                    hp[:, k, :],
                                    start=False,
                                    stop=(k == 2 and m != 0),
                                )

                        nc.scalar.activation(r_sb[:], rzv[:, 0:3], Sig)
                        nc.scalar.activation(z_sb[:], rzv[:, 3:6], Sig)
                        # chain: rhn -> npre -> tanh -> t1 -> h; zh rides the
                        # DVE queue between npre and t1 (its data is ready
                        # early, and it fills the tanh window).
                        nc.vector.tensor_tensor(
                            out=rhn[:], in0=hn_v, in1=r_sb[:], op=Mult
                        )
                        nc.vector.tensor_tensor(
                            out=npre[:], in0=rhn[:], in1=gi_n, op=Add
                        )
                        nc.vector.tensor_tensor(
                            out=zh[:], in0=z_sb[:], in1=hp[:], op=Mult
                        )
                        nc.scalar.activation(zm1[:], rzv[:, 3:6], Sig, scale=-1.0)
                        nc.scalar.activation(n_sb[:], npre[:], Tanh)
                        nc.vector.tensor_tensor(
                            out=t1[:], in0=zm1[:], in1=n_sb[:], op=Mult
                        )
                        nc.vector.tensor_tensor(
                            out=h_new[:], in0=zh[:], in1=t1[:], op=Add
                        )

                    h_list.append(h_new)
                    if t == ARITY - 1:
                        qsl = slice(ch * NCH // ARITY, (ch + 1) * NCH // ARITY)
                        # child-mean of final hiddens -> h0 of level 4
                        nc.vector.tensor_reduce(
                            out=csum5[:, :, qsl],
                            in_=h_new.rearrange("p j (q c) -> p j q c", c=ARITY),
                            axis=mybir.AxisListType.X,
                            op=Add,
                        )
                        # x4 = hacc + h (raw sum; /8 folded into wih_s)
                        hperm = h_new.rearrange("p j (q c) -> p j c q", c=ARITY)
                        for j in range(J):
                            nc.vector.tensor_tensor(
                                out=x4[:, j, ch],
                                in0=hacc5[:, j, sl].rearrange(
                                    "p (q c) -> p c q", c=ARITY
                                ),
                                in1=hperm[:, j],
                                op=Add,
                            )
                    else:
                        # parent-major contiguous accumulate. On the DVE,
                        # NOT gpsimd: gpsimd TTs here both queue behind the
                        # gather descgens (lib batching) and visibly slow
                        # concurrent DVE ops 2-3x while running.
                        nc.vector.tensor_tensor(
                            out=hacc5[:, :, sl],
                            in0=hacc5[:, :, sl],
                            in1=h_new[:],
                            op=Add,
                        )
                h_prev = h_list

        # ================= level 4: 64 parents, single chunk =================
        with nc.named_scope("level4"):
            h4 = gates.tile([P, J, P4], bf16, name="h4", tag="h4")
            nc.scalar.mul(h4[:], csum5[:], 1.0 / ARITY)

            # gi4 = x4 @ (w_ih/8).T + biases: one 3-bank group per role
            # (regions j' = bank, N=512 over all (child, parent) columns).
            # psum->sbuf copies split across scalar/vector so they drain in
            # ~2 copy-times, not 3 serial on the vector engine.
            # biases enter as per-partition adds on the psum->sbuf copies
            # (gi4 is feature-major, so the bias is constant per partition
            # within each m-tile) -- no bias matmuls on the PE. Roles r/z
            # run first (level-4 step 0 needs them for its id-MMs), each in
            # a ch0 phase (overlaps step 7 chunk 1) then a ch1 phase; the
            # n-role rides the ring afterwards.
            bias_col = {0: 0, 1: 3, 2: 9}

            def gi4_mms(role, rp, ch):
                for jj in range(3):
                    for k in range(J):
                        nc.tensor.matmul(
                            rp[:, jj, NCH * ch : NCH * (ch + 1)],
                            wih_sb[:, k, 3 * role + jj, :],
                            x4[:, k, ch].rearrange("p c q -> p (c q)"),
                            start=(ch == 0 and k == 0),
                            stop=(k == 2),
                        )

            def gi4_copies(role, rp):
                for m in range(3):
                    src = rp[:, m, :].rearrange(
                        "p (ch c q) -> p ch c q", ch=2, c=ARITY
                    )
                    dst = gi4_roles[role][:, m].rearrange(
                        "p c (ch q) -> p ch c q", ch=2
                    )
                    bcol = bias_col[role] + m
                    if (role + m) % 2 == 0:
                        nc.scalar.add(
                            out=dst, in_=src, add=bias_sb[:, bcol : bcol + 1]
                        )
                    else:
                        nc.vector.tensor_scalar_add(
                            out=dst,
                            in0=src,
                            scalar1=bias_sb[:, bcol : bcol + 1],
                        )

            rzpA, _ = rz_tile()
            rzpB, _ = rz_tile()
            for ch in (0, 1):
                gi4_mms(0, rzpA, ch)
                gi4_mms(1, rzpB, ch)
            gi4_copies(0, rzpA)
            gi4_copies(1, rzpB)
            rzpC, _ = rz_tile()
            for ch in (0, 1):
                gi4_mms(2, rzpC, ch)
            gi4_copies(2, rzpC)

            for t in range(ARITY):
                c = ARITY - 1 - t
                gi_n = gi4_roles[2][:, :, c, :]

                r_sb = gates.tile([P, J, P4], bf16, name="r4", tag="r4")
                z_sb = gates.tile([P, J, P4], bf16, name="z4", tag="z4")
                rhn = gates.tile([P, J, P4], bf16, name="rhn4", tag="rhn4")
                npre = gates.tile([P, J, P4], bf16, name="npre4", tag="npre4")
                n_sb = gates.tile([P, J, P4], bf16, name="nsb4", tag="nsb4")
                t1 = gates.tile([P, J, P4], bf16, name="t14", tag="t14")
                zh4 = gates.tile([P, J, P4], bf16, name="zh4", tag="zh4")
                zm14 = gates.tile([P, J, P4], bf16, name="zm14", tag="zm14")
                h4n = gates.tile([P, J, P4], bf16, name="h4", tag="h4")

                # r, hn, and z each fill their OWN psum tile instance so
                # readers wait only their own writers: sig(r) fires after the
                # 10 r-side MMs, not the whole 30-MM burst. id-MMs first
                # (h-independent), then hh r, hn, hh z.
                rzpA, _ = rz_tile()
                rv = rzpA[:, 0, :].rearrange("p (m n) -> p m n", m=8)[:, 0:3]
                nc.tensor.matmul(
                    rzpA[:, 0, : 3 * P4].rearrange("p (a n) -> p a n", a=3),
                    ident[:],
                    gi4_roles[0][:, :, c, :],
                    start=True,
                    stop=False,
                )
                for m in range(J):
                    for k in range(J):
                        nc.tensor.matmul(
                            rv[:, m],
                            whh_sb[:, k, m, :],
                            h4[:, k, :],
                            start=False,
                            stop=(m == 2 and k == 2),
                        )
                hnp, _ = hn_tile()
                hn_v = hnp[:, 0, :].rearrange("p (m n) -> p m n", m=8)[:, 0:3]
                nc.tensor.matmul(
                    hnp[:, 0, : 3 * P4],
                    biasmm_sb[:, 3, :],
                    onehot4_sb[:, :, :].rearrange("k m n -> k (m n)"),
                    start=True,
                    stop=False,
                )
                for m in range(J):
                    for k in range(J):
                        nc.tensor.matmul(
                            hn_v[:, m],
                            whh_sb[:, k, 6 + m, :],
                            h4[:, k, :],
                            start=False,
                            stop=(m == 2 and k == 2),
                        )
                rzpB, _ = rz_tile()
                zv = rzpB[:, 0, :].rearrange("p (m n) -> p m n", m=8)[:, 0:3]
                nc.tensor.matmul(
                    rzpB[:, 0, : 3 * P4].rearrange("p (a n) -> p a n", a=3),
                    ident[:],
                    gi4_roles[1][:, :, c, :],
                    start=True,
                    stop=False,
                )
                for m in range(J):
                    for k in range(J):
                        nc.tensor.matmul(
                            zv[:, m],
                            whh_sb[:, k, 3 + m, :],
                            h4[:, k, :],
                            start=False,
                            stop=(m == 2 and k == 2),
                        )

                nc.scalar.activation(r_sb[:], rv, Sig)
                nc.scalar.activation(z_sb[:], zv, Sig)
                nc.vector.tensor_tensor(out=rhn[:], in0=hn_v, in1=r_sb[:], op=Mult)
                nc.vector.tensor_tensor(out=npre[:], in0=rhn[:], in1=gi_n, op=Add)
                nc.vector.tensor_tensor(out=zh4[:], in0=z_sb[:], in1=h4[:], op=Mult)
                nc.scalar.activation(zm14[:], zv, Sig, scale=-1.0)
                nc.scalar.activation(n_sb[:], npre[:], Tanh)
                nc.vector.tensor_tensor(out=t1[:], in0=zm14[:], in1=n_sb[:], op=Mult)
                nc.vector.tensor_tensor(out=h4n[:], in0=zh4[:], in1=t1[:], op=Add)
                # hacc off-chain on the DVE; with no gpsimd ops anywhere
                # after the gathers, the standard-ucode lib reload never
                # happens at all
                nc.vector.tensor_tensor(
                    out=hacc4[:], in0=hacc4[:], in1=h4n[:], op=Add
                )
                h4 = h4n

            # ---- ship level-4 outputs ----
            nc.sync.dma_start(out_x[:], hacc4[:])
            nc.sync.dma_start(out_h[:], h4[:])


def _build_program():
    if "nc" in _PROG_CACHE:
        return _PROG_CACHE["nc"]
    import concourse.bacc as bacc
    import concourse.mybir as mybir
    import concourse.tile as tile

    f32 = mybir.dt.float32
    bf16 = mybir.dt.bfloat16

    nc = bacc.Bacc(
        "TRN2",
        target_bir_lowering=False,
        debug=False,
        enable_asserts=False,
        num_devices=NCORES,
        num_swdge_queues=4,
    )
    gtab = nc.dram_tensor("gtab", [VOCAB, 9 * P], bf16, kind="ExternalInput").ap()
    idxs = nc.dram_tensor(
        "idxs", [P, 2 * ARITY, NCH // 16], mybir.dt.int16, kind="ExternalInput"
    ).ap()
    ident_in = nc.dram_tensor(
        "ident", [P, P + 3 * NCH], bf16, kind="ExternalInput"
    ).ap()
    whh_t = nc.dram_tensor("whh_t", [P, J, 9, P], bf16, kind="ExternalInput").ap()
    wih_s = nc.dram_tensor("wih_s", [P, J, 9, P], bf16, kind="ExternalInput").ap()
    biases = nc.dram_tensor("biases", [P, 12], f32, kind="ExternalInput").ap()
    threes = nc.dram_tensor("threes", [3, 3008], bf16, kind="ExternalInput").ap()
    out_x = nc.dram_tensor("out_x", [P, J, P4], f32, kind="ExternalOutput").ap()
    out_h = nc.dram_tensor("out_h", [P, J, P4], bf16, kind="ExternalOutput").ap()

    with tile.TileContext(nc) as tc:
        _emit(
            tc,
            nc,
            (gtab, idxs, ident_in, whh_t, wih_s, biases, threes, out_x, out_h),
        )
    nc.compile()
    _PROG_CACHE["nc"] = nc
    return nc


def _retile_weights(w):
    # w: [1152, 384] -> lhsT tiles [128(k_part), 3(k), 9(m), 128(m_col)] bf16
    wt = np.ascontiguousarray(w.T)  # [384, 1152]
    wt = wt.reshape(J, P, 9, P).transpose(1, 0, 2, 3)
    return np.ascontiguousarray(wt).astype(BF16)


def _prep_bias(b_ih, b_hh):
    biases = np.zeros((P, 12), np.float32)
    comb = (b_ih + b_hh).reshape(9, P)
    biases[:, 0:6] = comb[0:6].T
    biases[:, 6:9] = b_hh.reshape(9, P)[6:9].T
    biases[:, 9:12] = b_ih.reshape(9, P)[6:9].T
    return biases


def _prep_biasmm(b_ih, b_hh):
    # biasmm[k, role, p]: roles 0..2 feed the gi4 bias matmuls (combined r/z
    # bias and b_ih_n); role 3 is b_hh_n for the hn-bank bias matmuls.
    bias12 = np.zeros((P, 12), np.float32)
    comb = (b_ih + b_hh).reshape(9, P)
    bias12[:, 0:6] = comb[0:6].T
    bias12[:, 6:9] = b_ih.reshape(9, P)[6:9].T
    bias12[:, 9:12] = b_hh.reshape(9, P)[6:9].T
    out = bias12.T.reshape(4, 3, P).transpose(1, 0, 2)
    return np.ascontiguousarray(out).astype(BF16)


def _prep_onehot(n):
    out = np.zeros((3, 3, n), np.float32)
    for k in range(3):
        out[k, k, :] = 1.0
    return out.astype(BF16)


def _prep_idxs(tokens_core):
    # child-major half-gathers: gather (g, h) covers child 7-g, parents
    # [256h, 256h+256), wrapped [16, 16] (idx i -> [i%16, i//16]) and
    # replicated over the 8 Q7 stripes.
    byc = tokens_core.reshape(P5, ARITY)  # [parent, child]
    out = np.zeros((P, 2 * ARITY, NCH // 16), np.int16)
    for g in range(ARITY):
        c = ARITY - 1 - g
        for h in range(2):
            seg = byc[h * NCH : (h + 1) * NCH, c]
            wrapped = seg.reshape(NCH // 16, 16).T.astype(np.int16)
            out[:, 2 * g + h, :] = np.tile(wrapped, (8, 1))
    return out


def _gru_step_batch(x_t, h, w_ih, w_hh, b_ih, b_hh):
    gi = x_t @ w_ih.T + b_ih
    gh = h @ w_hh.T + b_hh
    i_r, i_z, i_n = np.split(gi, 3, axis=-1)
    h_r, h_z, h_n = np.split(gh, 3, axis=-1)
    r = 1.0 / (1.0 + np.exp(-(i_r + h_r)))
    z = 1.0 / (1.0 + np.exp(-(i_z + h_z)))
    n = np.tanh(i_n + r * h_n)
    return (1.0 - z) * n + z * h


def _host_level(xs, h0, w_ih, w_hh, b_ih, b_hh):
    # xs: [n_parents, ARITY, D] child outputs in natural child order.
    h = h0
    acc = np.zeros_like(h)
    for t in range(ARITY):
        h = _gru_step_batch(xs[:, ARITY - 1 - t], h, w_ih, w_hh, b_ih, b_hh)
        acc += h
    return acc / ARITY, h


def kernel(leaf_tokens, embed_table, w_ih, w_hh, b_ih, b_hh):
    from concourse.bass_utils import run_bass_kernel_spmd

    leaf_tokens = np.asarray(leaf_tokens, np.int32)
    embed_table = np.asarray(embed_table, np.float32)
    w_ih = np.asarray(w_ih, np.float32)
    w_hh = np.asarray(w_hh, np.float32)
    b_ih = np.asarray(b_ih, np.float32)
    b_hh = np.asarray(b_hh, np.float32)

    nc = _build_program()

    # G = embed @ w_ih.T with r/z biases (b_ih+b_hh) and n bias (b_ih) folded.
    bias_fold = np.concatenate([(b_ih + b_hh)[: 2 * DIM], b_ih[2 * DIM :]])
    G = embed_table @ w_ih.T
    G += bias_fold
    G = G.astype(BF16)

    whh_t = _retile_weights(w_hh)
    wih_s = _retile_weights(w_ih / ARITY)
    biases = _prep_bias(b_ih, b_hh)
    identity = np.zeros((P, P + 3 * NCH), np.float32)
    identity[:, 0:P] = np.eye(P, dtype=np.float32)
    bhn = b_hh[2 * DIM :].reshape(3, P)  # [m, p]
    identity[:, P:] = np.repeat(bhn.T[:, :, None], NCH, axis=2).reshape(P, 3 * NCH)
    identity = identity.astype(BF16)
    threes = np.zeros((3, 3008), BF16)
    threes[:, 0:512] = _prep_biasmm(b_ih, b_hh).reshape(3, 512)
    threes[:, 512:1280] = _prep_onehot(NCH).reshape(3, 768)
    threes[:, 1280:2816] = _prep_onehot(512).reshape(3, 1536)
    threes[:, 2816:3008] = _prep_onehot(P4).reshape(3, 192)

    in_maps = []
    for core in range(NCORES):
        toks = leaf_tokens[core * LEAVES_CORE : (core + 1) * LEAVES_CORE]
        in_maps.append(
            {
                "gtab": G,
                "idxs": _prep_idxs(toks),
                "ident": identity,
                "whh_t": whh_t,
                "wih_s": wih_s,
                "biases": biases,
                "threes": threes,
            }
        )
    res = run_bass_kernel_spmd(nc, in_maps, core_ids=list(range(NCORES)))

    # ---- host epilogue: levels 3, 2 (per core) and the root ----
    w_ih64 = w_ih.astype(np.float64)
    w_hh64 = w_hh.astype(np.float64)
    b_ih64 = b_ih.astype(np.float64)
    b_hh64 = b_hh.astype(np.float64)

    x3 = np.zeros((NCORES, P4, DIM), np.float64)
    h4 = np.zeros((NCORES, P4, DIM), np.float64)
    for core in range(NCORES):
        hacc = np.asarray(res.results[core]["out_x"], np.float64)  # [128,3,64]
        hh = np.asarray(
            res.results[core]["out_h"].astype(np.float32), np.float64
        )  # [128,3,64]
        # [p, k, node] -> node-major [node, feat=128k+p]
        x3[core] = hacc.transpose(2, 1, 0).reshape(P4, DIM) / ARITY
        h4[core] = hh.transpose(2, 1, 0).reshape(P4, DIM)

    # level 3: 8 parents per core
    xs3 = x3.reshape(NCORES * ARITY, ARITY, DIM)
    h03 = h4.reshape(NCORES * ARITY, ARITY, DIM).mean(axis=1)
    x2, h3 = _host_level(xs3, h03, w_ih64, w_hh64, b_ih64, b_hh64)
    # level 2: 1 parent per core
    xs2 = x2.reshape(NCORES, ARITY, DIM)
    h02 = h3.reshape(NCORES, ARITY, DIM).mean(axis=1)
    x1, h2 = _host_level(xs2, h02, w_ih64, w_hh64, b_ih64, b_hh64)
    # root: 1 node over the 8 cores' outputs
    xs1 = x1.reshape(1, ARITY, DIM)
    h01 = h2.reshape(1, ARITY, DIM).mean(axis=1)
    x0, _ = _host_level(xs1, h01, w_ih64, w_hh64, b_ih64, b_hh64)

    return x0.astype(np.float32).reshape(1, 1, DIM)

